# revision 1
# baseline (speedup 1.0000x reference)
"""FCPlanenet Trainium2 kernel (8-core data-parallel over batch).

Network (per batch of T=8192 points, feature-major [feat, T] on chip):
  net0 = p @ Wpos + bpos            [256, T]   (K=3 matmul, quad-packed)
  net1 = relu(net0) @ W0 + b0       [128, T]   (K=256)
  netk+1 = relu(cat(netk, max_t netk)) @ Wk + bk   for W1..W3
  out = MLP head over max_t net4    [9] per batch

Key restructuring: concat-with-pooled splits into
  Wk_a.T @ relu(netk)  +  (Wk_b.T @ relu(pooled_k) + bk)   <- per-feature const vec
The const vec is applied at PSUM->SBUF drain time.  DVE drains store the
offset form  r~ = max(psum, -vec) = relu(net) - vec  via tensor_tensor_reduce,
which also emits the per-chunk max for the pooling reduction in the same pass.
The offset is folded into the next layer's const vec (two vec chains: R for
true-relu chunks, T for offset chunks; chunk classes are fixed across layers).
"""

import os
import sys

import numpy as np

for _p in ("/opt/trn_rl_repo", "/root/.axon_site/_ro/trn_rl_repo"):
    if os.path.isdir(_p) and _p not in sys.path:
        sys.path.insert(0, _p)

import concourse.bass as bass  # noqa: E402
import concourse.tile as tile  # noqa: E402
from concourse import bacc, mybir  # noqa: E402
from concourse.bass_utils import run_bass_kernel_spmd  # noqa: E402

F32 = mybir.dt.float32
F32R = mybir.dt.float32r
AX = mybir.AxisListType.X
MAX = mybir.AluOpType.max
RELU = mybir.ActivationFunctionType.Relu
IDENT = mybir.ActivationFunctionType.Identity

NCORES = 8
B = 32
T = 8192
BPC = B // NCORES  # batches per core
NQ = 4             # point quads (for K=3 matmul row-packing)
QT = T // NQ       # 2048 points per quad
NCH = 512          # matmul free-dim chunk (one PSUM bank)
NSUP = 1024        # drain supertile (2 chunks)
NST = T // NSUP    # 8 supertiles per layer

# bias tile columns
BC_BPOS_A, BC_BPOS_B = 0, 1
BC_B0, BC_B1, BC_B2, BC_B3 = 2, 3, 4, 5
BC_BC, BC_BM0, BC_BM1, BC_BM2, BC_BP = 6, 7, 8, 9, 10

# wm tile blocks of 128 cols: w0a w0b w1a w1b w2a w2b w3a w3b wc wm0 wm1 wm2 wp
WM_COLS = 13 * 128 + 16

# pos supertiles drained on DVE instead of ACT (engine balance): (s, h) keys
POS_DVE = ((3, 0),)

# All drains store true relu on ACT; all pooling maxes are DVE reduces read
# directly from PSUM (pre-bias).  The per-layer constant vector vec_k (bias +
# pooled-path contribution) is added at drain time; the pooled max is
# recovered as m_k = relu(max_t(psum_k) + vec_k).
NSUP2 = 1024           # drain/reduce supertile (2 banks)
NST2 = T // NSUP2      # 8 supertiles per layer


def _f32r(ap):
    return ap if ap.dtype == F32R else ap.bitcast(F32R)


def _f32(ap):
    return ap if ap.dtype == F32 else ap.bitcast(F32)


def _emit_core_program(tc, nc, pt_d, wpos_d, wm_d, bias_d, out_d, reps=1):
    ctx_pools = []

    def pool(name, bufs, space="SBUF"):
        p = tc.alloc_tile_pool(name=name, bufs=bufs, space=space)
        ctx_pools.append(p)
        return p

    const = pool("const", 1)
    ptp = pool("ptp", 2)
    r0p = pool("r0p", 1)
    netp = pool("netp", 2)
    smallp = pool("smallp", 8)
    vecp = pool("vecp", 30)
    headp = pool("headp", 2)
    psmm = pool("psmm", 3, space="PSUM")
    psvp = pool("psvp", 2, space="PSUM")

    # ---- constants ----
    wpos_sb = const.tile([99, 256], F32R, name="wpos_sb")
    for q in range(NQ):
        nc.sync.dma_start(wpos_sb[32 * q:32 * q + 3, :], wpos_d[q])
    bias_sb = const.tile([128, 16], F32, name="bias_sb")
    nc.sync.dma_start(bias_sb[:], bias_d)
    wm_sb = const.tile([128, WM_COLS], F32R, name="wm_sb")
    _wm_loaded = [False]

    def load_wm():
        if not _wm_loaded[0]:
            nc.sync.dma_start(wm_sb[:, 0:256], wm_d[:, 0:256])
            nc.sync.dma_start(wm_sb[:, 256:], wm_d[:, 256:])
            _wm_loaded[0] = True

    def W(i):
        return wm_sb[:, 128 * i:128 * (i + 1)]

    def bcol(i):
        return bias_sb[:, i:i + 1]

    def pos_tasks(b, pt_sb, r0):
        """16 supertile tasks for the pos layer of batch b.  A couple go to
        DVE (dual-op relu) to balance engine load."""
        tasks = []
        for qp in range(2):
            for s in range(4):
                for h in range(2):
                    def t(qp=qp, s=s, h=h):
                        ps = psmm.tile([128, NSUP2], F32, tag="mm", name="ps_pos")
                        for jq in range(2):
                            q = 2 * qp + jq
                            nc.tensor.matmul(
                                ps[:, NCH * jq:NCH * (jq + 1)],
                                wpos_sb[32 * q:32 * q + 3, 128 * h:128 * (h + 1)],
                                pt_sb[32 * q:32 * q + 3, NCH * s:NCH * (s + 1)],
                                start=True, stop=True,
                                tile_position=(32 * q, 0),
                            )
                        g0 = 8 * qp + s
                        dst = (r0[h].rearrange("p (g c) -> p g c", c=NCH)
                               [:, g0:g0 + 5:4, :])
                        srcv = ps.rearrange("p (g c) -> p g c", c=NCH)
                        if (s, h) in POS_DVE:
                            nc.vector.tensor_scalar(dst, srcv, bcol(BC_BPOS_A + h),
                                                    0.0, op0=mybir.AluOpType.add,
                                                    op1=MAX)
                        else:
                            nc.scalar.activation(dst, srcv, RELU,
                                                 bias=bcol(BC_BPOS_A + h))
                    tasks.append(t)
        return tasks

    def layer_tasks(li, b, r0, r_prev, r_out, part):
        """4 supertile tasks for pooled layer li (0..2)."""
        tasks = []
        vec = None  # bound at emit time via vec_box
        for st in range(NST2):
            def t(st=st, li=li):
                ps = psmm.tile([128, NSUP2], F32, tag="mm", name=f"ps_l{li}")
                for j in range(2):
                    c = 2 * st + j
                    osl = ps[:, NCH * j:NCH * (j + 1)]
                    csl = slice(NCH * c, NCH * (c + 1))
                    if li == 0:
                        nc.tensor.matmul(osl, W(0), r0[0][:, csl],
                                         start=True, stop=False)
                        nc.tensor.matmul(osl, W(1), r0[1][:, csl],
                                         start=False, stop=True)
                    else:
                        nc.tensor.matmul(osl, W(2 * li), r_prev[:, csl],
                                         start=True, stop=True)
                dsl = slice(NSUP2 * st, NSUP2 * (st + 1))
                nc.scalar.activation(r_out[:, dsl], ps[:], RELU, bias=vec_box[0])
                if st >= 6:
                    # last two: single-width reduces chase their own drains so
                    # the boundary chain isn't gated by a [2048] reduce
                    nc.vector.reduce_max(part[:, st - 3:st - 2],
                                         _f32(r_out[:, dsl]), axis=AX)
                elif st % 2 == 1:
                    psl = slice(NSUP2 * (st - 1), NSUP2 * (st + 1))
                    nc.vector.reduce_max(part[:, st // 2:st // 2 + 1],
                                         _f32(r_out[:, psl]), axis=AX)
            tasks.append(t)
        return tasks

    def d_tasks(b, r_prev, partD, scratch=None):
        """D-stage supertiles.  With scratch (last batch): ACT copies the psum
        out and DVE reduces pairs from SBUF -- fills the tail's idle ACT."""
        tasks = []
        for st in range(NST2):
            def t(st=st):
                ps = psmm.tile([128, NSUP2], F32, tag="mm", name="ps_d")
                for j in range(2):
                    c = 2 * st + j
                    csl = slice(NCH * c, NCH * (c + 1))
                    nc.tensor.matmul(ps[:, NCH * j:NCH * (j + 1)], W(6),
                                     r_prev[:, csl], start=True, stop=True)
                if scratch is None:
                    nc.vector.reduce_max(partD[:, st:st + 1], ps[:], axis=AX)
                else:
                    dsl = slice(NSUP2 * st, NSUP2 * (st + 1))
                    nc.scalar.copy(scratch[:, dsl], ps[:])
                    if st % 2 == 1:
                        psl = slice(NSUP2 * (st - 1), NSUP2 * (st + 1))
                        nc.vector.reduce_max(partD[:, st // 2:st // 2 + 1],
                                             _f32(scratch[:, psl]), axis=AX)
            tasks.append(t)
        return tasks

    def boundary_pre(li, part, b):
        # part cols hold maxes of relu(net) (post-bias SBUF values), so the
        # pooled relu max is just their max -- no further bias/relu needed.
        m_cur = vecp.tile([128, 1], F32, tag="v", name=f"m{li}_{b}")
        nc.vector.reduce_max(m_cur, part[:, 0:5], axis=AX)
        return m_cur

    def boundary_post(li, m_cur, b):
        psv = psvp.tile([128, 1], F32, tag="psv", name=f"psv{li}_{b}")
        nc.tensor.matmul(psv[:], _f32(W(2 * li + 3)), _f32(m_cur),
                         start=True, stop=True)
        vec_next = vecp.tile([128, 1], F32, tag="v", name=f"vec{li + 2}_{b}")
        nc.vector.tensor_scalar_add(vec_next, psv[:], bcol(BC_B1 + li))
        vec_box[0] = vec_next

    def interleave(a, bl):
        out = []
        n = max(len(a), len(bl))
        for i in range(n):
            if i < len(a):
                out.append(a[i])
            if i < len(bl):
                out.append(bl[i])
        return out

    import contextlib

    def _rep_scope():
        if reps > 1:
            return tc.For_i(0, reps, 1,
                            hint_engines=(mybir.EngineType.PE,
                                          mybir.EngineType.Activation,
                                          mybir.EngineType.DVE))
        return contextlib.nullcontext()

    with _rep_scope():
        vec_box = [None]

        # per-batch state created lazily
        def new_batch_state(b):
            pt_sb = ptp.tile([99, QT], F32R, tag="pt", name="pt_sb")
            for q in range(NQ):
                nc.sync.dma_start(pt_sb[32 * q:32 * q + 3, :], pt_d[b, q])
            r0a = r0p.tile([128, T], F32R, tag="r0a", name="r0a")
            r0b = r0p.tile([128, T], F32R, tag="r0b", name="r0b")
            return pt_sb, (r0a, r0b)

        # prologue: pos(0) interleaved with L0(0) in chunk-ready order
        st0 = new_batch_state(0)
        load_wm()
        states = {0: st0}
        p0 = pos_tasks(0, st0[0], st0[1])
        vec_box[0] = bcol(BC_B0)
        r1_0 = netp.tile([128, T], F32R, tag="net", name="r1_0")
        part0_0 = smallp.tile([128, NST2], F32, tag="pp", name="p0_0")
        l0_0 = layer_tasks(0, 0, st0[1], None, r1_0, part0_0)
        for t in p0[0:4]:
            t()
        l0_0[0](); l0_0[2]()
        for t in p0[4:8]:
            t()
        l0_0[1](); l0_0[3]()
        for t in p0[8:12]:
            t()
        l0_0[4](); l0_0[6]()
        for t in p0[12:16]:
            t()
        l0_0[5](); l0_0[7]()

        for b in range(BPC):
            _, r0 = states[b]

            if b == 0:
                r1, part0 = r1_0, part0_0
            else:
                vec_box[0] = bcol(BC_B0)
                r1 = netp.tile([128, T], F32R, tag="net", name=f"r1_{b}")
                part0 = smallp.tile([128, NST2], F32, tag="pp", name=f"p0_{b}")
                for t in layer_tasks(0, b, r0, None, r1, part0):
                    t()
            m0 = boundary_pre(0, part0, b)
            filler = []
            if b + 1 < BPC:
                stn = new_batch_state(b + 1)
                states[b + 1] = stn
                filler = pos_tasks(b + 1, stn[0], stn[1])
            for t in filler[0:2]:
                t()
            boundary_post(0, m0, b)

            r2 = netp.tile([128, T], F32R, tag="net", name=f"r2_{b}")
            part1 = smallp.tile([128, NST2], F32, tag="pp", name=f"p1_{b}")
            for t in layer_tasks(1, b, None, r1, r2, part1):
                t()
            m1 = boundary_pre(1, part1, b)
            for t in filler[2:4]:
                t()
            boundary_post(1, m1, b)

            r3 = netp.tile([128, T], F32R, tag="net", name=f"r3_{b}")
            part2 = smallp.tile([128, NST2], F32, tag="pp", name=f"p2_{b}")
            for t in interleave(layer_tasks(2, b, None, r2, r3, part2), filler[4:8]):
                t()
            m2 = boundary_pre(2, part2, b)
            partD = smallp.tile([128, NST2], F32, tag="pp", name=f"pD_{b}")
            if b + 1 < BPC:
                dts = d_tasks(b, r3, partD)
                npart = NST2
            else:
                scr = netp.tile([128, T], F32, tag="net", name="d_scr")
                dts = d_tasks(b, r3, partD, scratch=scr)
                npart = NST2 // 2
            for t in interleave(dts, filler[8:16]):
                t()
            boundary_post(2, m2, b)
            pmaxD = vecp.tile([128, 1], F32, tag="v", name=f"pmaxD_{b}")
            nc.vector.reduce_max(pmaxD, partD[:, 0:npart], axis=AX)
            s_b = vecp.tile([128, 1], F32, tag="v", name=f"s_{b}")
            nc.vector.tensor_scalar(s_b, pmaxD, vec_box[0], 0.0,
                                    op0=mybir.AluOpType.add, op1=MAX)
            # incremental head for this batch (all tiny fp32 ops)
            hb = s_b
            for wi, bi in ((8, BC_BC), (9, BC_BM0), (10, BC_BM1), (11, BC_BM2)):
                ps = psvp.tile([128, 1], F32, tag="psv", name=f"ph{wi}_{b}")
                nc.tensor.matmul(ps[:], _f32(W(wi)), hb[:], start=True, stop=True)
                h2 = vecp.tile([128, 1], F32, tag="v", name=f"h{wi}_{b}")
                nc.vector.tensor_scalar(h2, ps[:], bcol(bi), 0.0,
                                        op0=mybir.AluOpType.add, op1=MAX)
                hb = h2
            ps9 = psvp.tile([9, 1], F32, tag="psv", name=f"po_{b}")
            nc.tensor.matmul(ps9[:], _f32(wm_sb[:, 1536:1536 + 9]), hb[:],
                             start=True, stop=True)
            ob = headp.tile([9, 1], F32, tag="o", name=f"ob_{b}")
            nc.scalar.activation(ob, ps9[:], IDENT, bias=bias_sb[0:9, BC_BP:BC_BP + 1])
            nc.sync.dma_start(out_d[b:b + 1].rearrange("b f -> f b"), ob[:])

    for p in reversed(ctx_pools):
        p.release()


def build_program(reps=1):
    nc = bacc.Bacc("TRN2", target_bir_lowering=False, debug=False,
                   num_devices=NCORES)
    pt_d = nc.dram_tensor("pt", [BPC, NQ, 3, QT], F32R, kind="ExternalInput").ap()
    wpos_d = nc.dram_tensor("wpos", [NQ, 3, 256], F32R, kind="ExternalInput").ap()
    wm_d = nc.dram_tensor("wm", [128, WM_COLS], F32R, kind="ExternalInput").ap()
    bias_d = nc.dram_tensor("bias", [128, 16], F32, kind="ExternalInput").ap()
    out_d = nc.dram_tensor("out", [BPC, 9], F32, kind="ExternalOutput").ap()
    with tile.TileContext(nc) as tc:
        _emit_core_program(tc, nc, pt_d, wpos_d, wm_d, bias_d, out_d, reps=reps)
    nc.compile()
    return nc


def prepare_host_inputs(inputs):
    """Shared (weights) and per-core (points) host-side packing."""
    f = lambda k: np.ascontiguousarray(np.asarray(inputs[k], np.float32))
    p = f("p")
    W_pos, b_pos = f("W_pos"), f("b_pos")
    W0, b0 = f("W0"), f("b0")
    W1, b1 = f("W1"), f("b1")
    W2, b2 = f("W2"), f("b2")
    W3, b3 = f("W3"), f("b3")
    Wc, bc = f("Wc"), f("bc")
    Wm0, bm0 = f("Wm0"), f("bm0")
    Wm1, bm1 = f("Wm1"), f("bm1")
    Wm2, bm2 = f("Wm2"), f("bm2")
    Wp, bp = f("Wp"), f("bp")

    wpos = np.broadcast_to(W_pos, (NQ, 3, 256)).copy()  # replicated per quad

    wm = np.zeros((128, WM_COLS), np.float32)
    blocks = [W0[:128], W0[128:], W1[:128], W1[128:], W2[:128], W2[128:],
              W3[:128], W3[128:], Wc, Wm0, Wm1, Wm2]
    for i, blk in enumerate(blocks):
        wm[:, 128 * i:128 * (i + 1)] = blk
    wm[:, 1536:1536 + 9] = Wp

    bias = np.zeros((128, 16), np.float32)
    bias[:, BC_BPOS_A] = b_pos[:128]
    bias[:, BC_BPOS_B] = b_pos[128:]
    bias[:, BC_B0] = b0
    bias[:, BC_B1] = b1
    bias[:, BC_B2] = b2
    bias[:, BC_B3] = b3
    bias[:, BC_BC] = bc
    bias[:, BC_BM0] = bm0
    bias[:, BC_BM1] = bm1
    bias[:, BC_BM2] = bm2
    bias[:9, BC_BP] = bp

    shared = {"wpos": wpos, "wm": wm, "bias": bias}

    in_maps = []
    for core in range(NCORES):
        pc = p[core * BPC:(core + 1) * BPC]          # [BPC, T, 3]
        pt = np.empty((BPC, NQ, 3, QT), np.float32)
        for b in range(BPC):
            for q in range(NQ):
                pt[b, q] = pc[b, q * QT:(q + 1) * QT, :].T
        in_maps.append({"pt": pt, **shared})
    return in_maps


_PROGRAM_CACHE = {}


def kernel(**inputs):
    reps = 1
    if reps not in _PROGRAM_CACHE:
        _PROGRAM_CACHE[reps] = build_program(reps)
    nc = _PROGRAM_CACHE[reps]
    in_maps = prepare_host_inputs(inputs)
    res = run_bass_kernel_spmd(nc, in_maps, core_ids=list(range(NCORES)))
    out = np.concatenate([res.results[i]["out"] for i in range(NCORES)], axis=0)
    return out.astype(np.float32)



# revision 8
# speedup vs baseline: 1.0870x; 1.0870x over previous
"""FCPlanenet Trainium2 kernel (8-core data-parallel over batch).

Network (per batch of T=8192 points, feature-major [feat, T] on chip):
  net0 = p @ Wpos + bpos            [256, T]   (K=3 matmul, quad-packed)
  net1 = relu(net0) @ W0 + b0       [128, T]   (K=256)
  netk+1 = relu(cat(netk, max_t netk)) @ Wk + bk   for W1..W3
  out = MLP head over max_t net4    [9] per batch

The pooled-concat half is rank-1 (same vector at every point), so each layer
reduces to Wk_a.T @ relu(netk) plus a per-feature constant vector C applied
at drain time.  Big matmuls run in bf16 (weights + activations; ~1e-3 final
rel err, gate is 2e-2).  PSUM-touching elementwise work is split between two
engines (gpsimd cannot access PSUM):
  - ACT:  true-relu drains  r = relu(psum + C_R)          (R-class chunks)
          plus identity drains of some D-stage psums to bf16 scratch
  - DVE:  tensor_scalar+accum drains  r~ = max(psum, -C_T)  (= relu - C_T)
          emitting the per-chunk pooled max in the same pass (T-class)
Pooling coverage of ACT-drained bf16 data costs almost nothing: one DVE
tensor_scalar+accum in 4x mode (0.26 ns/elem, all-SBUF 2-byte).  gpsimd takes
the tiny [128,1] boundary constant ops.  Offset-class constants fold into the
next layer's constants via tiny matvecs (Wa @ C_T, off the critical path).
"""

import os
import sys

import numpy as np

for _p in ("/opt/trn_rl_repo", "/root/.axon_site/_ro/trn_rl_repo"):
    if os.path.isdir(_p) and _p not in sys.path:
        sys.path.insert(0, _p)

import concourse.bass as bass  # noqa: E402
import concourse.tile as tile  # noqa: E402
from concourse import bacc, mybir  # noqa: E402
from concourse.bass_utils import run_bass_kernel_spmd  # noqa: E402

F32 = mybir.dt.float32
F32R = mybir.dt.float32r
BF16 = mybir.dt.bfloat16
AX = mybir.AxisListType.X
MAX = mybir.AluOpType.max
ADD = mybir.AluOpType.add
MUL = mybir.AluOpType.mult
RELU = mybir.ActivationFunctionType.Relu
IDENT = mybir.ActivationFunctionType.Identity

NCORES = 8
B = 32
T = 8192
BPC = B // NCORES  # batches per core
NQ = 4             # point quads (for K=3 matmul row-packing)
QT = T // NQ       # 2048 points per quad
NCH = 512          # matmul free-dim chunk (one PSUM bank)
NSUP = 1024        # drain supertile (2 chunks)
NST = T // NSUP    # 8 supertiles per layer

NEG_INF = -1.0e30

# bias tile columns
BC_BPOS_A, BC_BPOS_B = 0, 1
BC_B0, BC_B1, BC_B2, BC_B3 = 2, 3, 4, 5
BC_BC, BC_BM0, BC_BM1, BC_BM2, BC_BP = 6, 7, 8, 9, 10
BC_NEG_B0 = 11

# wm tile blocks of 128 cols: w0a w0b w1a w1b w2a w2b w3a w3b wc wm0 wm1 wm2 wp
WM_COLS = 13 * 128 + 16

# ---- engine assignment tables (tuning knobs) ----
# L-layer supertiles 0..7: 'A' = ACT true-relu (R-class, bf16 out, covered by
# one DVE 4x accum pass), 'V' = DVE ts+accum offset form (T-class).
L_ENG = ("A", "A", "A", "V", "V", "V", "V", "V")
NR = 3                       # leading R-class st count (contiguous)
L_ACC_T = {3: 1, 4: 2, 5: 3, 6: 4, 7: 5}
L_NT = 5  # number of T accum cols (starting at col 1)

# pos supertile tasks idx=8*qp+2*s+h: 'A' = ACT, 'V' = DVE (true relu both)
POS_ENG = ("A",) * 16

# D stage: sts in D_ACT drain via ACT-ident to bf16 scratch (one DVE 4x
# accum covers them); the rest are DVE ts+accum psum singles.
# r3 chunk classes: sts 0..NR-1 are R (need C_R^D), the rest T (C_T^D).
D_ACT = (3, 4)               # must be same-class (T) and contiguous
D_COL = {0: 0, 1: 1, 2: 2, "act": 3, 5: 4, 6: 5, 7: 6}
D_RCOL = (0, 3)              # accD cols [0,3) are R-class raw maxes
D_TCOL = (3, 7)              # accD cols [3,7) are T-class raw maxes


def _f32r(ap):
    return ap if ap.dtype == F32R else ap.bitcast(F32R)


def _f32(ap):
    return ap if ap.dtype == F32 else ap.bitcast(F32)


def _emit_core_program(tc, nc, pt_d, wpos_d, wm16_d, wmf_d, bias_d, out_d,
                       reps=1):
    ctx_pools = []

    def pool(name, bufs, space="SBUF"):
        p = tc.alloc_tile_pool(name=name, bufs=bufs, space=space)
        ctx_pools.append(p)
        return p

    const = pool("const", 1)
    ptp = pool("ptp", 2)
    r0p = pool("r0p", 1)
    netp = pool("netp", 2)
    smallp = pool("smallp", 8)
    vecp = pool("vecp", 30)
    covp = pool("covp", 2)
    dscp = pool("dscp", 2)
    headp = pool("headp", 2)
    psmm = pool("psmm", 3, space="PSUM")
    psvp = pool("psvp", 2, space="PSUM")

    # ---- constants ----
    wpos_sb = const.tile([99, 256], BF16, name="wpos_sb")
    for q in range(NQ):
        nc.sync.dma_start(wpos_sb[32 * q:32 * q + 3, :], wpos_d[q])
    bias_sb = const.tile([128, 16], F32, name="bias_sb")
    nc.sync.dma_start(bias_sb[:], bias_d)
    wm_sb = const.tile([128, WM_COLS], BF16, name="wm_sb")
    wmf_sb = const.tile([128, WM_COLS], F32, name="wmf_sb")
    _wm_loaded = [False]

    def load_wm():
        if not _wm_loaded[0]:
            nc.sync.dma_start(wm_sb[:, 0:WM_COLS], wm16_d[:, 0:WM_COLS])
            nc.sync.dma_start(wmf_sb[:, 0:WM_COLS], wmf_d[:, 0:WM_COLS])
            _wm_loaded[0] = True

    def W(i):       # bf16 weights for the big matmuls
        return wm_sb[:, 128 * i:128 * (i + 1)]

    def Wf(i):      # f32 weights for [128,1] matvecs
        return wmf_sb[:, 128 * i:128 * (i + 1)]

    def bcol(i):
        return bias_sb[:, i:i + 1]

    def mk_acc(name):
        """Accum tile, initialized to -inf: the HW tensor_scalar accum_out
        read-modify-writes the destination."""
        acc = smallp.tile([128, 8], F32, tag="pp", name=name)
        nc.gpsimd.memset(acc[:], NEG_INF)
        return acc

    def pos_tasks(b, pt_sb, r0):
        """16 supertile tasks for the pos layer of batch b (true relu)."""
        tasks = []
        for qp in range(2):
            for s in range(4):
                for h in range(2):
                    def t(qp=qp, s=s, h=h):
                        ps = psmm.tile([128, NSUP], F32, tag="mm", name="ps_pos")
                        for jq in range(2):
                            q = 2 * qp + jq
                            nc.tensor.matmul(
                                ps[:, NCH * jq:NCH * (jq + 1)],
                                wpos_sb[32 * q:32 * q + 3, 128 * h:128 * (h + 1)],
                                pt_sb[32 * q:32 * q + 3, NCH * s:NCH * (s + 1)],
                                start=True, stop=True,
                                tile_position=(32 * q, 0),
                            )
                        g0 = 8 * qp + s
                        dst = (r0[h].rearrange("p (g c) -> p g c", c=NCH)
                               [:, g0:g0 + 5:4, :])
                        srcv = ps.rearrange("p (g c) -> p g c", c=NCH)
                        idx = 8 * qp + 2 * s + h
                        if POS_ENG[idx] == "V":
                            nc.vector.tensor_scalar(dst, srcv, bcol(BC_BPOS_A + h),
                                                    0.0, op0=ADD, op1=MAX)
                        else:
                            nc.scalar.activation(dst, srcv, RELU,
                                                 bias=bcol(BC_BPOS_A + h))
                    tasks.append(t)
        return tasks

    def layer_tasks(li, b, r0, r_prev, r_out, acc, consts):
        """Supertile tasks for pooled layer li (0..2).  consts = (cR, cT, negT)
        access thunks (bound at emit time).  acc: [128,8] f32 accum tile.
        After the last R drain one DVE 4x pass covers their pooling max."""
        cR, cT, negT = consts
        tasks = []

        def emit_st(st, li):
            ps = psmm.tile([128, NSUP], F32, tag="mm", name=f"ps_l{li}")
            for j in range(2):
                c = 2 * st + j
                osl = ps[:, NCH * j:NCH * (j + 1)]
                csl = slice(NCH * c, NCH * (c + 1))
                if li == 0:
                    nc.tensor.matmul(osl, W(0), r0[0][:, csl],
                                     start=True, stop=False)
                    nc.tensor.matmul(osl, W(1), r0[1][:, csl],
                                     start=False, stop=True)
                else:
                    nc.tensor.matmul(osl, W(2 * li), r_prev[:, csl],
                                     start=True, stop=True)
            dsl = slice(NSUP * st, NSUP * (st + 1))
            if L_ENG[st] == "A":
                nc.scalar.activation(r_out[:, dsl], ps[:], RELU, bias=cR())
                if st == NR - 1:
                    cov = covp.tile([128, NR * NSUP], BF16, tag="cov",
                                    name="cov")
                    nc.vector.tensor_scalar(cov[:], r_out[:, 0:NR * NSUP],
                                            NEG_INF, NEG_INF, op0=MAX, op1=MAX,
                                            accum_out=acc[:, 0:1])
            else:
                col = L_ACC_T[st]
                nc.vector.tensor_scalar(r_out[:, dsl], ps[:], negT(), NEG_INF,
                                        op0=MAX, op1=MAX,
                                        accum_out=acc[:, col:col + 1])

        for st in range(NST):
            tasks.append(lambda st=st, li=li: emit_st(st, li))
        return tasks

    def d_tasks(b, r_prev, accD):
        """D-stage supertiles: matmuls + raw psum maxes into accD."""
        tasks = []
        dscr = dscp.tile([128, len(D_ACT) * NSUP], BF16, tag="dsc",
                         name=f"dscr_{b}")

        def emit_st(st):
            ps = psmm.tile([128, NSUP], F32, tag="mm", name="ps_d")
            for j in range(2):
                c = 2 * st + j
                csl = slice(NCH * c, NCH * (c + 1))
                nc.tensor.matmul(ps[:, NCH * j:NCH * (j + 1)], W(6),
                                 r_prev[:, csl], start=True, stop=True)
            if st in D_ACT:
                k = D_ACT.index(st)
                nc.scalar.activation(dscr[:, NSUP * k:NSUP * (k + 1)], ps[:],
                                     IDENT, bias=0.0)
                if st == D_ACT[-1]:
                    cov = covp.tile([128, len(D_ACT) * NSUP], BF16, tag="cov",
                                    name="covd")
                    col = D_COL["act"]
                    nc.vector.tensor_scalar(cov[:], dscr[:], NEG_INF, NEG_INF,
                                            op0=MAX, op1=MAX,
                                            accum_out=accD[:, col:col + 1])
            else:
                col = D_COL[st]
                scrj = headp.tile([128, NSUP], BF16, tag="scrj", name="scrj")
                nc.vector.tensor_scalar(scrj[:], ps[:], NEG_INF, NEG_INF,
                                        op0=MAX, op1=MAX,
                                        accum_out=accD[:, col:col + 1])

        for st in range(NST):
            tasks.append(lambda st=st: emit_st(st))
        return tasks

    def interleave(a, bl):
        out = []
        n = max(len(a), len(bl))
        for i in range(n):
            if i < len(a):
                out.append(a[i])
            if i < len(bl):
                out.append(bl[i])
        return out

    import contextlib

    def _rep_scope():
        if reps > 1:
            return tc.For_i(0, reps, 1,
                            hint_engines=(mybir.EngineType.PE,
                                          mybir.EngineType.Activation,
                                          mybir.EngineType.DVE,
                                          mybir.EngineType.Pool))
        return contextlib.nullcontext()

    with _rep_scope():
        # per-batch state created lazily
        def new_batch_state(b):
            pt_sb = ptp.tile([99, QT], BF16, tag="pt", name="pt_sb")
            for q in range(NQ):
                nc.sync.dma_start(pt_sb[32 * q:32 * q + 3, :], pt_d[b, q])
            r0a = r0p.tile([128, T], BF16, tag="r0a", name="r0a")
            r0b = r0p.tile([128, T], BF16, tag="r0b", name="r0b")
            return pt_sb, (r0a, r0b)

        def mk_consts_L0():
            # C_R = C_T = b0 (pos chunks are all true-relu)
            return (lambda: bcol(BC_B0), lambda: bcol(BC_B0),
                    lambda: bcol(BC_NEG_B0))

        def boundary(li, b, acc, cur):
            """After layer li (0..2): compute m and next-layer constants from
            the finished layer's consts `cur`.  Next layer li+1 uses
            Wa=W(2*(li+1)), Wb=W(2*(li+1)+1); for li==2 the "next layer" is
            the D stage (Wa=W(6), Wb=W(7), bias b3).
            acc col 0 = max_t relu(net) over R-chunks (final form);
            acc cols 1..L_NT = max(max_t psum, -C_T) over T-chunks."""
            wa_i = 2 * (li + 1)
            wb_i = 2 * (li + 1) + 1
            bc_i = BC_B1 + li
            cR_cur, cT_cur = cur[0](), cur[1]()
            # pooled max m = max(reduce(accT) + C_T, accR)
            mT = vecp.tile([128, 1], F32, tag="v", name=f"mT{li}_{b}")
            nc.vector.tensor_reduce(mT, acc[:, 1:1 + L_NT], AX, MAX)
            mTc = vecp.tile([128, 1], F32, tag="v", name=f"mTc{li}_{b}")
            nc.gpsimd.tensor_scalar(mTc, mT, cT_cur, 0.0, op0=ADD, op1=ADD)
            m = vecp.tile([128, 1], F32, tag="v", name=f"m{li}_{b}")
            nc.vector.tensor_tensor(m, mTc, acc[:, 0:1], MAX)
            # psv2 = Wa @ cT_cur (off critical path w.r.t. m)
            psv2 = psvp.tile([128, 1], F32, tag="psv", name=f"psv2_{li}_{b}")
            nc.tensor.matmul(psv2[:], Wf(wa_i), cT_cur, start=True, stop=True)
            # psv = Wb @ m
            psv = psvp.tile([128, 1], F32, tag="psv", name=f"psv_{li}_{b}")
            nc.tensor.matmul(psv[:], Wf(wb_i), m, start=True, stop=True)
            cR = vecp.tile([128, 1], F32, tag="v", name=f"cR{li}_{b}")
            nc.scalar.activation(cR, psv[:], IDENT, bias=bcol(bc_i))
            cT = vecp.tile([128, 1], F32, tag="v", name=f"cT{li}_{b}")
            nc.vector.tensor_tensor(cT, cR, psv2[:], ADD)
            negT = vecp.tile([128, 1], F32, tag="v", name=f"nT{li}_{b}")
            nc.gpsimd.tensor_scalar(negT, cT, -1.0, 0.0, op0=MUL, op1=ADD)
            consts = (lambda: cR, lambda: cT, lambda: negT)
            return consts, cT

        # prologue: pos(0) interleaved with L0(0) in chunk-ready order
        st0 = new_batch_state(0)
        load_wm()
        states = {0: st0}
        p0 = pos_tasks(0, st0[0], st0[1])
        consts0 = mk_consts_L0()
        r1_0 = netp.tile([128, T], BF16, tag="net", name="r1_0")
        acc0_0 = mk_acc("a0_0")
        l0_0 = layer_tasks(0, 0, st0[1], None, r1_0, acc0_0, consts0)
        for t in p0[0:4]:
            t()
        l0_0[0](); l0_0[2]()
        for t in p0[4:8]:
            t()
        l0_0[1](); l0_0[3]()
        for t in p0[8:12]:
            t()
        l0_0[4](); l0_0[6]()
        for t in p0[12:16]:
            t()
        l0_0[5](); l0_0[7]()

        for b in range(BPC):
            _, r0 = states[b]

            if b == 0:
                r1, acc0 = r1_0, acc0_0
            else:
                consts0 = mk_consts_L0()
                r1 = netp.tile([128, T], BF16, tag="net", name=f"r1_{b}")
                acc0 = mk_acc(f"a0_{b}")
                for t in layer_tasks(0, b, r0, None, r1, acc0, consts0):
                    t()

            filler = []
            if b + 1 < BPC:
                stn = new_batch_state(b + 1)
                states[b + 1] = stn
                filler = pos_tasks(b + 1, stn[0], stn[1])
            for t in filler[0:2]:
                t()
            consts1, cT1 = boundary(0, b, acc0, consts0)

            r2 = netp.tile([128, T], BF16, tag="net", name=f"r2_{b}")
            acc1 = mk_acc(f"a1_{b}")
            for t in interleave(layer_tasks(1, b, None, r1, r2, acc1, consts1),
                                filler[2:5]):
                t()
            consts2, cT2 = boundary(1, b, acc1, consts1)

            r3 = netp.tile([128, T], BF16, tag="net", name=f"r3_{b}")
            acc2 = mk_acc(f"a2_{b}")
            for t in interleave(layer_tasks(2, b, None, r2, r3, acc2, consts2),
                                filler[5:10]):
                t()
            constsD, cTD = boundary(2, b, acc2, consts2)
            cRD = constsD[0]()

            accD = mk_acc(f"aD_{b}")
            for t in interleave(d_tasks(b, r3, accD), filler[10:16]):
                t()

            # s = relu(max(reduce(accD_T) + C_T^D, reduce(accD_R) + C_R^D))
            sT0 = vecp.tile([128, 1], F32, tag="v", name=f"sT0_{b}")
            nc.vector.tensor_reduce(sT0, accD[:, D_TCOL[0]:D_TCOL[1]], AX, MAX)
            sR0 = vecp.tile([128, 1], F32, tag="v", name=f"sR0_{b}")
            nc.vector.tensor_reduce(sR0, accD[:, D_RCOL[0]:D_RCOL[1]], AX, MAX)
            sT = vecp.tile([128, 1], F32, tag="v", name=f"sT_{b}")
            nc.gpsimd.tensor_scalar(sT, sT0, cTD, 0.0, op0=ADD, op1=ADD)
            sR = vecp.tile([128, 1], F32, tag="v", name=f"sR_{b}")
            nc.gpsimd.tensor_scalar(sR, sR0, cRD, 0.0, op0=ADD, op1=ADD)
            spre = vecp.tile([128, 1], F32, tag="v", name=f"sp_{b}")
            nc.vector.tensor_tensor(spre, sT, sR, MAX)
            s_b = vecp.tile([128, 1], F32, tag="v", name=f"s_{b}")
            nc.gpsimd.tensor_scalar(s_b, spre, 0.0, 0.0, op0=MAX, op1=ADD)

            # incremental head for this batch (all tiny fp32 ops)
            hb = s_b
            for wi, bi in ((8, BC_BC), (9, BC_BM0), (10, BC_BM1), (11, BC_BM2)):
                ps = psvp.tile([128, 1], F32, tag="psv", name=f"ph{wi}_{b}")
                nc.tensor.matmul(ps[:], Wf(wi), hb[:], start=True, stop=True)
                h2 = vecp.tile([128, 1], F32, tag="v", name=f"h{wi}_{b}")
                nc.scalar.activation(h2, ps[:], RELU, bias=bcol(bi))
                hb = h2
            ps9 = psvp.tile([9, 1], F32, tag="psv", name=f"po_{b}")
            nc.tensor.matmul(ps9[:], wmf_sb[:, 1536:1536 + 9], hb[:],
                             start=True, stop=True)
            ob = headp.tile([9, 1], F32, tag="o", name=f"ob_{b}")
            nc.scalar.activation(ob, ps9[:], IDENT, bias=bias_sb[0:9, BC_BP:BC_BP + 1])
            nc.sync.dma_start(out_d[b:b + 1].rearrange("b f -> f b"), ob[:])

    for p in reversed(ctx_pools):
        p.release()


def build_program(reps=1):
    nc = bacc.Bacc("TRN2", target_bir_lowering=False, debug=False,
                   num_devices=NCORES)
    pt_d = nc.dram_tensor("pt", [BPC, NQ, 3, QT], BF16, kind="ExternalInput").ap()
    wpos_d = nc.dram_tensor("wpos", [NQ, 3, 256], BF16, kind="ExternalInput").ap()
    wm16_d = nc.dram_tensor("wm16", [128, WM_COLS], BF16, kind="ExternalInput").ap()
    wmf_d = nc.dram_tensor("wmf", [128, WM_COLS], F32, kind="ExternalInput").ap()
    bias_d = nc.dram_tensor("bias", [128, 16], F32, kind="ExternalInput").ap()
    out_d = nc.dram_tensor("out", [BPC, 9], F32, kind="ExternalOutput").ap()
    with tile.TileContext(nc) as tc:
        _emit_core_program(tc, nc, pt_d, wpos_d, wm16_d, wmf_d, bias_d, out_d,
                           reps=reps)
    nc.compile()
    return nc


def prepare_host_inputs(inputs):
    """Shared (weights) and per-core (points) host-side packing."""
    import ml_dtypes
    BF = ml_dtypes.bfloat16
    f = lambda k: np.ascontiguousarray(np.asarray(inputs[k], np.float32))
    p = f("p")
    W_pos, b_pos = f("W_pos"), f("b_pos")
    W0, b0 = f("W0"), f("b0")
    W1, b1 = f("W1"), f("b1")
    W2, b2 = f("W2"), f("b2")
    W3, b3 = f("W3"), f("b3")
    Wc, bc = f("Wc"), f("bc")
    Wm0, bm0 = f("Wm0"), f("bm0")
    Wm1, bm1 = f("Wm1"), f("bm1")
    Wm2, bm2 = f("Wm2"), f("bm2")
    Wp, bp = f("Wp"), f("bp")

    wpos = np.broadcast_to(W_pos, (NQ, 3, 256)).copy()  # replicated per quad

    wm = np.zeros((128, WM_COLS), np.float32)
    blocks = [W0[:128], W0[128:], W1[:128], W1[128:], W2[:128], W2[128:],
              W3[:128], W3[128:], Wc, Wm0, Wm1, Wm2]
    for i, blk in enumerate(blocks):
        wm[:, 128 * i:128 * (i + 1)] = blk
    wm[:, 1536:1536 + 9] = Wp

    bias = np.zeros((128, 16), np.float32)
    bias[:, BC_BPOS_A] = b_pos[:128]
    bias[:, BC_BPOS_B] = b_pos[128:]
    bias[:, BC_B0] = b0
    bias[:, BC_B1] = b1
    bias[:, BC_B2] = b2
    bias[:, BC_B3] = b3
    bias[:, BC_BC] = bc
    bias[:, BC_BM0] = bm0
    bias[:, BC_BM1] = bm1
    bias[:, BC_BM2] = bm2
    bias[:9, BC_BP] = bp
    bias[:, BC_NEG_B0] = -b0

    shared = {"wpos": wpos.astype(BF), "wm16": wm.astype(BF),
              "wmf": wm, "bias": bias}

    in_maps = []
    for core in range(NCORES):
        pc = p[core * BPC:(core + 1) * BPC]          # [BPC, T, 3]
        pt = np.empty((BPC, NQ, 3, QT), np.float32)
        for b in range(BPC):
            for q in range(NQ):
                pt[b, q] = pc[b, q * QT:(q + 1) * QT, :].T
        in_maps.append({"pt": pt.astype(BF), **shared})
    return in_maps


_PROGRAM_CACHE = {}


def kernel(**inputs):
    reps = 1
    if reps not in _PROGRAM_CACHE:
        _PROGRAM_CACHE[reps] = build_program(reps)
    nc = _PROGRAM_CACHE[reps]
    in_maps = prepare_host_inputs(inputs)
    res = run_bass_kernel_spmd(nc, in_maps, core_ids=list(range(NCORES)))
    out = np.concatenate([res.results[i]["out"] for i in range(NCORES)], axis=0)
    return out.astype(np.float32)


# revision 9
# speedup vs baseline: 1.2217x; 1.1238x over previous
"""FCPlanenet Trainium2 kernel (8-core data-parallel over batch).

Network (per batch of T=8192 points, feature-major [feat, T] on chip):
  net0 = p @ Wpos + bpos            [256, T]   (K=3 matmul, quad-packed)
  net1 = relu(net0) @ W0 + b0       [128, T]   (K=256)
  netk+1 = relu(cat(netk, max_t netk)) @ Wk + bk   for W1..W3
  out = MLP head over max_t net4    [9] per batch

The pooled-concat half is rank-1 (same vector at every point), so each layer
reduces to Wk_a.T @ relu(netk) plus a per-feature constant vector C applied
at drain time.  Big matmuls run in bf16 (weights + activations; ~1e-3 final
rel err, gate is 2e-2).  PSUM-touching elementwise work is split between two
engines (gpsimd cannot access PSUM):
  - ACT:  true-relu drains  r = relu(psum + C_R)          (R-class chunks)
          plus identity drains of some D-stage psums to bf16 scratch
  - DVE:  tensor_scalar+accum drains  r~ = max(psum, -C_T)  (= relu - C_T)
          emitting the per-chunk pooled max in the same pass (T-class)
Pooling coverage of ACT-drained bf16 data costs almost nothing: one DVE
tensor_scalar+accum in 4x mode (0.26 ns/elem, all-SBUF 2-byte).  gpsimd takes
the tiny [128,1] boundary constant ops.  Offset-class constants fold into the
next layer's constants via tiny matvecs (Wa @ C_T, off the critical path).
"""

import os
import sys

import numpy as np

for _p in ("/opt/trn_rl_repo", "/root/.axon_site/_ro/trn_rl_repo"):
    if os.path.isdir(_p) and _p not in sys.path:
        sys.path.insert(0, _p)

import concourse.bass as bass  # noqa: E402
import concourse.tile as tile  # noqa: E402
from concourse import bacc, mybir  # noqa: E402
from concourse.bass_utils import run_bass_kernel_spmd  # noqa: E402

F32 = mybir.dt.float32
F32R = mybir.dt.float32r
BF16 = mybir.dt.bfloat16
AX = mybir.AxisListType.X
MAX = mybir.AluOpType.max
ADD = mybir.AluOpType.add
MUL = mybir.AluOpType.mult
RELU = mybir.ActivationFunctionType.Relu
IDENT = mybir.ActivationFunctionType.Identity

NCORES = 8
B = 32
T = 8192
BPC = B // NCORES  # batches per core
NQ = 4             # point quads (for K=3 matmul row-packing)
QT = T // NQ       # 2048 points per quad
NCH = 512          # matmul free-dim chunk (one PSUM bank)
NSUP = 1024        # drain supertile (2 chunks)
NST = T // NSUP    # 8 supertiles per layer

NEG_INF = -1.0e30

# bias tile columns
BC_BPOS_A, BC_BPOS_B = 0, 1
BC_B0, BC_B1, BC_B2, BC_B3 = 2, 3, 4, 5
BC_BC, BC_BM0, BC_BM1, BC_BM2, BC_BP = 6, 7, 8, 9, 10
BC_NEG_B0 = 11

# wm tile blocks of 128 cols: w0a w0b w1a w1b w2a w2b w3a w3b wc wm0 wm1 wm2 wp
WM_COLS = 13 * 128 + 16

# ---- engine assignment tables (tuning knobs) ----
# L-layer supertiles 0..7: 'A' = ACT true-relu (R-class, bf16 out, covered by
# one DVE 4x accum pass), 'V' = DVE ts+accum offset form (T-class).
# R-sts sit at stride 3 so ACT and DVE drain work interleaves in time and
# the R-chunk 4x cov pass can still use one regular strided AP.
L_ENG = ("A", "V", "V", "A", "V", "V", "A", "V")
L_RSTS = (0, 3, 6)           # R-class sts (stride 3), cov -> acc col 0
L_ACC_T = {1: 1, 2: 2, 4: 3, 5: 4, 7: 5}
L_NT = 5  # number of T accum cols (starting at col 1)

# pos supertile tasks idx=8*qp+2*s+h: 'A' = ACT, 'V' = DVE (true relu both).
# Batch 0 has no other work to overlap, so its pos spreads onto DVE too.
POS_ENG = ("A",) * 16
POS_ENG_PRO = tuple("V" if i in (2, 5, 8, 11, 13, 15) else "A"
                    for i in range(16))

# D stage: sts in D_ACT drain via ACT-ident to bf16 scratch (one DVE 4x
# accum covers them); the rest are DVE ts+accum psum singles.
# r3 chunk classes follow L_RSTS: sts 0,3,6 are R (C_R^D), rest T (C_T^D).
D_ACT = (4, 5)               # must be same-class (T) and contiguous
D_COL = {0: 0, 3: 1, 6: 2, 1: 3, 2: 4, "act": 5, 7: 6}
D_RCOL = (0, 3)              # accD cols [0,3) are R-class raw maxes
D_TCOL = (3, 7)              # accD cols [3,7) are T-class raw maxes


def _f32r(ap):
    return ap if ap.dtype == F32R else ap.bitcast(F32R)


def _f32(ap):
    return ap if ap.dtype == F32 else ap.bitcast(F32)


def _emit_core_program(tc, nc, pt_d, wpos_d, wm16_d, wmf_d, bias_d, out_d,
                       reps=1):
    ctx_pools = []

    def pool(name, bufs, space="SBUF"):
        p = tc.alloc_tile_pool(name=name, bufs=bufs, space=space)
        ctx_pools.append(p)
        return p

    const = pool("const", 1)
    ptp = pool("ptp", 2)
    r0p = pool("r0p", 1)
    netp = pool("netp", 2)
    smallp = pool("smallp", 8)
    vecp = pool("vecp", 30)
    covp = pool("covp", 2)
    dscp = pool("dscp", 2)
    headp = pool("headp", 2)
    psmm = pool("psmm", 3, space="PSUM")
    psvp = pool("psvp", 2, space="PSUM")

    # ---- constants ----
    wpos_sb = const.tile([99, 256], BF16, name="wpos_sb")
    for q in range(NQ):
        nc.sync.dma_start(wpos_sb[32 * q:32 * q + 3, :], wpos_d[q])
    bias_sb = const.tile([128, 16], F32, name="bias_sb")
    nc.sync.dma_start(bias_sb[:], bias_d)
    wm_sb = const.tile([128, WM_COLS], BF16, name="wm_sb")
    wmf_sb = const.tile([128, WM_COLS], F32, name="wmf_sb")
    _wm_loaded = [False]

    def load_wm():
        if not _wm_loaded[0]:
            nc.sync.dma_start(wm_sb[:, 0:WM_COLS], wm16_d[:, 0:WM_COLS])
            nc.sync.dma_start(wmf_sb[:, 0:WM_COLS], wmf_d[:, 0:WM_COLS])
            _wm_loaded[0] = True

    def W(i):       # bf16 weights for the big matmuls
        return wm_sb[:, 128 * i:128 * (i + 1)]

    def Wf(i):      # f32 weights for [128,1] matvecs
        return wmf_sb[:, 128 * i:128 * (i + 1)]

    def bcol(i):
        return bias_sb[:, i:i + 1]

    def mk_acc(name):
        """Accum tile, initialized to -inf: the HW tensor_scalar accum_out
        read-modify-writes the destination."""
        acc = smallp.tile([128, 8], F32, tag="pp", name=name)
        nc.gpsimd.memset(acc[:], NEG_INF)
        return acc

    def pos_tasks(b, pt_sb, r0):
        """16 supertile tasks for the pos layer of batch b (true relu)."""
        eng = POS_ENG_PRO if b == 0 else POS_ENG
        tasks = []
        for qp in range(2):
            for s in range(4):
                for h in range(2):
                    def t(qp=qp, s=s, h=h):
                        ps = psmm.tile([128, NSUP], F32, tag="mm", name="ps_pos")
                        for jq in range(2):
                            q = 2 * qp + jq
                            nc.tensor.matmul(
                                ps[:, NCH * jq:NCH * (jq + 1)],
                                wpos_sb[32 * q:32 * q + 3, 128 * h:128 * (h + 1)],
                                pt_sb[32 * q:32 * q + 3, NCH * s:NCH * (s + 1)],
                                start=True, stop=True,
                                tile_position=(32 * q, 0),
                            )
                        g0 = 8 * qp + s
                        dst = (r0[h].rearrange("p (g c) -> p g c", c=NCH)
                               [:, g0:g0 + 5:4, :])
                        srcv = ps.rearrange("p (g c) -> p g c", c=NCH)
                        idx = 8 * qp + 2 * s + h
                        if eng[idx] == "V":
                            nc.vector.tensor_scalar(dst, srcv, bcol(BC_BPOS_A + h),
                                                    0.0, op0=ADD, op1=MAX)
                        else:
                            nc.scalar.activation(dst, srcv, RELU,
                                                 bias=bcol(BC_BPOS_A + h))
                    tasks.append(t)
        return tasks

    def layer_tasks(li, b, r0, r_prev, r_out, acc, consts):
        """Supertile tasks for pooled layer li (0..2).  consts = (cR, cT, negT)
        access thunks (bound at emit time).  acc: [128,8] f32 accum tile.
        After the last R drain one DVE 4x pass covers their pooling max."""
        cR, cT, negT = consts
        tasks = []

        def emit_st(st, li):
            ps = psmm.tile([128, NSUP], F32, tag="mm", name=f"ps_l{li}")
            for j in range(2):
                c = 2 * st + j
                osl = ps[:, NCH * j:NCH * (j + 1)]
                csl = slice(NCH * c, NCH * (c + 1))
                if li == 0:
                    nc.tensor.matmul(osl, W(0), r0[0][:, csl],
                                     start=True, stop=False)
                    nc.tensor.matmul(osl, W(1), r0[1][:, csl],
                                     start=False, stop=True)
                else:
                    nc.tensor.matmul(osl, W(2 * li), r_prev[:, csl],
                                     start=True, stop=True)
            dsl = slice(NSUP * st, NSUP * (st + 1))
            if L_ENG[st] == "A":
                nc.scalar.activation(r_out[:, dsl], ps[:], RELU, bias=cR())
                if st == L_RSTS[-1]:
                    cov = covp.tile([128, len(L_RSTS) * NSUP], BF16, tag="cov",
                                    name="cov")
                    rsrc = (r_out.rearrange("p (g c) -> p g c", c=NSUP)
                            [:, L_RSTS[0]:L_RSTS[-1] + 1:3, :])
                    nc.vector.tensor_scalar(
                        cov.rearrange("p (g c) -> p g c", c=NSUP), rsrc,
                        NEG_INF, NEG_INF, op0=MAX, op1=MAX,
                        accum_out=acc[:, 0:1])
            else:
                col = L_ACC_T[st]
                nc.vector.tensor_scalar(r_out[:, dsl], ps[:], negT(), NEG_INF,
                                        op0=MAX, op1=MAX,
                                        accum_out=acc[:, col:col + 1])

        for st in range(NST):
            tasks.append(lambda st=st, li=li: emit_st(st, li))
        return tasks

    def d_tasks(b, r_prev, accD):
        """D-stage supertiles: matmuls + raw psum maxes into accD."""
        tasks = []
        dscr = dscp.tile([128, len(D_ACT) * NSUP], BF16, tag="dsc",
                         name=f"dscr_{b}")

        def emit_st(st):
            ps = psmm.tile([128, NSUP], F32, tag="mm", name="ps_d")
            for j in range(2):
                c = 2 * st + j
                csl = slice(NCH * c, NCH * (c + 1))
                nc.tensor.matmul(ps[:, NCH * j:NCH * (j + 1)], W(6),
                                 r_prev[:, csl], start=True, stop=True)
            if st in D_ACT:
                k = D_ACT.index(st)
                nc.scalar.activation(dscr[:, NSUP * k:NSUP * (k + 1)], ps[:],
                                     IDENT, bias=0.0)
                if st == D_ACT[-1]:
                    cov = covp.tile([128, len(D_ACT) * NSUP], BF16, tag="cov",
                                    name="covd")
                    col = D_COL["act"]
                    nc.vector.tensor_scalar(cov[:], dscr[:], NEG_INF, NEG_INF,
                                            op0=MAX, op1=MAX,
                                            accum_out=accD[:, col:col + 1])
            else:
                col = D_COL[st]
                scrj = headp.tile([128, NSUP], BF16, tag="scrj", name="scrj")
                nc.vector.tensor_scalar(scrj[:], ps[:], NEG_INF, NEG_INF,
                                        op0=MAX, op1=MAX,
                                        accum_out=accD[:, col:col + 1])

        for st in range(NST):
            tasks.append(lambda st=st: emit_st(st))
        return tasks

    def interleave(a, bl):
        out = []
        n = max(len(a), len(bl))
        for i in range(n):
            if i < len(a):
                out.append(a[i])
            if i < len(bl):
                out.append(bl[i])
        return out

    import contextlib

    def _rep_scope():
        if reps > 1:
            return tc.For_i(0, reps, 1,
                            hint_engines=(mybir.EngineType.PE,
                                          mybir.EngineType.Activation,
                                          mybir.EngineType.DVE,
                                          mybir.EngineType.Pool))
        return contextlib.nullcontext()

    with _rep_scope():
        # per-batch state created lazily
        def new_batch_state(b):
            pt_sb = ptp.tile([99, QT], BF16, tag="pt", name="pt_sb")
            for q in range(NQ):
                nc.sync.dma_start(pt_sb[32 * q:32 * q + 3, :], pt_d[b, q])
            r0a = r0p.tile([128, T], BF16, tag="r0a", name="r0a")
            r0b = r0p.tile([128, T], BF16, tag="r0b", name="r0b")
            return pt_sb, (r0a, r0b)

        def mk_consts_L0():
            # C_R = C_T = b0 (pos chunks are all true-relu)
            return (lambda: bcol(BC_B0), lambda: bcol(BC_B0),
                    lambda: bcol(BC_NEG_B0))

        def boundary(li, b, acc, cur):
            """After layer li (0..2): compute m and next-layer constants from
            the finished layer's consts `cur`.  Next layer li+1 uses
            Wa=W(2*(li+1)), Wb=W(2*(li+1)+1); for li==2 the "next layer" is
            the D stage (Wa=W(6), Wb=W(7), bias b3).
            acc col 0 = max_t relu(net) over R-chunks (final form);
            acc cols 1..L_NT = max(max_t psum, -C_T) over T-chunks."""
            wa_i = 2 * (li + 1)
            wb_i = 2 * (li + 1) + 1
            bc_i = BC_B1 + li
            cR_cur, cT_cur = cur[0](), cur[1]()
            # pooled max m = max(reduce(accT) + C_T, accR)
            mT = vecp.tile([128, 1], F32, tag="v", name=f"mT{li}_{b}")
            nc.vector.tensor_reduce(mT, acc[:, 1:1 + L_NT], AX, MAX)
            mTc = vecp.tile([128, 1], F32, tag="v", name=f"mTc{li}_{b}")
            nc.gpsimd.tensor_scalar(mTc, mT, cT_cur, 0.0, op0=ADD, op1=ADD)
            m = vecp.tile([128, 1], F32, tag="v", name=f"m{li}_{b}")
            nc.vector.tensor_tensor(m, mTc, acc[:, 0:1], MAX)
            # psv2 = Wa @ cT_cur (off critical path w.r.t. m)
            psv2 = psvp.tile([128, 1], F32, tag="psv", name=f"psv2_{li}_{b}")
            nc.tensor.matmul(psv2[:], Wf(wa_i), cT_cur, start=True, stop=True)
            # psv = Wb @ m
            psv = psvp.tile([128, 1], F32, tag="psv", name=f"psv_{li}_{b}")
            nc.tensor.matmul(psv[:], Wf(wb_i), m, start=True, stop=True)
            cR = vecp.tile([128, 1], F32, tag="v", name=f"cR{li}_{b}")
            nc.scalar.activation(cR, psv[:], IDENT, bias=bcol(bc_i))
            negT = vecp.tile([128, 1], F32, tag="v", name=f"nT{li}_{b}")
            nc.vector.scalar_tensor_tensor(negT, cR, -1.0, psv2[:],
                                           op0=MUL, op1=mybir.AluOpType.subtract)
            cT = vecp.tile([128, 1], F32, tag="v", name=f"cT{li}_{b}")
            nc.gpsimd.tensor_scalar(cT, negT, -1.0, 0.0, op0=MUL, op1=ADD)
            consts = (lambda: cR, lambda: cT, lambda: negT)
            return consts, cT

        # prologue: pos(0) interleaved with L0(0) in chunk-ready order
        st0 = new_batch_state(0)
        load_wm()
        states = {0: st0}
        p0 = pos_tasks(0, st0[0], st0[1])
        consts0 = mk_consts_L0()
        r1_0 = netp.tile([128, T], BF16, tag="net", name="r1_0")
        acc0_0 = mk_acc("a0_0")
        l0_0 = layer_tasks(0, 0, st0[1], None, r1_0, acc0_0, consts0)
        for t in p0[0:4]:
            t()
        l0_0[0](); l0_0[2]()
        for t in p0[4:8]:
            t()
        l0_0[1](); l0_0[3]()
        for t in p0[8:12]:
            t()
        l0_0[4](); l0_0[6]()
        for t in p0[12:16]:
            t()
        l0_0[5](); l0_0[7]()

        for b in range(BPC):
            _, r0 = states[b]

            if b == 0:
                r1, acc0 = r1_0, acc0_0
            else:
                consts0 = mk_consts_L0()
                r1 = netp.tile([128, T], BF16, tag="net", name=f"r1_{b}")
                acc0 = mk_acc(f"a0_{b}")
                for t in layer_tasks(0, b, r0, None, r1, acc0, consts0):
                    t()

            filler = []
            if b + 1 < BPC:
                stn = new_batch_state(b + 1)
                states[b + 1] = stn
                filler = pos_tasks(b + 1, stn[0], stn[1])
            for t in filler[0:2]:
                t()
            consts1, cT1 = boundary(0, b, acc0, consts0)

            r2 = netp.tile([128, T], BF16, tag="net", name=f"r2_{b}")
            acc1 = mk_acc(f"a1_{b}")
            for t in interleave(layer_tasks(1, b, None, r1, r2, acc1, consts1),
                                filler[2:5]):
                t()
            consts2, cT2 = boundary(1, b, acc1, consts1)

            r3 = netp.tile([128, T], BF16, tag="net", name=f"r3_{b}")
            acc2 = mk_acc(f"a2_{b}")
            for t in interleave(layer_tasks(2, b, None, r2, r3, acc2, consts2),
                                filler[5:10]):
                t()
            constsD, cTD = boundary(2, b, acc2, consts2)
            cRD = constsD[0]()

            accD = mk_acc(f"aD_{b}")
            for t in interleave(d_tasks(b, r3, accD), filler[10:16]):
                t()

            # s = relu(max(reduce(accD_T) + C_T^D, reduce(accD_R) + C_R^D))
            sT0 = vecp.tile([128, 1], F32, tag="v", name=f"sT0_{b}")
            nc.vector.tensor_reduce(sT0, accD[:, D_TCOL[0]:D_TCOL[1]], AX, MAX)
            sR0 = vecp.tile([128, 1], F32, tag="v", name=f"sR0_{b}")
            nc.vector.tensor_reduce(sR0, accD[:, D_RCOL[0]:D_RCOL[1]], AX, MAX)
            sT = vecp.tile([128, 1], F32, tag="v", name=f"sT_{b}")
            nc.gpsimd.tensor_scalar(sT, sT0, cTD, 0.0, op0=ADD, op1=ADD)
            sR = vecp.tile([128, 1], F32, tag="v", name=f"sR_{b}")
            nc.gpsimd.tensor_scalar(sR, sR0, cRD, 0.0, op0=ADD, op1=ADD)
            spre = vecp.tile([128, 1], F32, tag="v", name=f"sp_{b}")
            nc.vector.tensor_tensor(spre, sT, sR, MAX)
            s_b = vecp.tile([128, 1], F32, tag="v", name=f"s_{b}")
            nc.gpsimd.tensor_scalar(s_b, spre, 0.0, 0.0, op0=MAX, op1=ADD)

            # incremental head for this batch (all tiny fp32 ops)
            hb = s_b
            for wi, bi in ((8, BC_BC), (9, BC_BM0), (10, BC_BM1), (11, BC_BM2)):
                ps = psvp.tile([128, 1], F32, tag="psv", name=f"ph{wi}_{b}")
                nc.tensor.matmul(ps[:], Wf(wi), hb[:], start=True, stop=True)
                h2 = vecp.tile([128, 1], F32, tag="v", name=f"h{wi}_{b}")
                nc.scalar.activation(h2, ps[:], RELU, bias=bcol(bi))
                hb = h2
            ps9 = psvp.tile([9, 1], F32, tag="psv", name=f"po_{b}")
            nc.tensor.matmul(ps9[:], wmf_sb[:, 1536:1536 + 9], hb[:],
                             start=True, stop=True)
            ob = headp.tile([9, 1], F32, tag="o", name=f"ob_{b}")
            nc.scalar.activation(ob, ps9[:], IDENT, bias=bias_sb[0:9, BC_BP:BC_BP + 1])
            nc.sync.dma_start(out_d[b:b + 1].rearrange("b f -> f b"), ob[:])

    for p in reversed(ctx_pools):
        p.release()


def build_program(reps=1):
    nc = bacc.Bacc("TRN2", target_bir_lowering=False, debug=False,
                   num_devices=NCORES)
    pt_d = nc.dram_tensor("pt", [BPC, NQ, 3, QT], BF16, kind="ExternalInput").ap()
    wpos_d = nc.dram_tensor("wpos", [NQ, 3, 256], BF16, kind="ExternalInput").ap()
    wm16_d = nc.dram_tensor("wm16", [128, WM_COLS], BF16, kind="ExternalInput").ap()
    wmf_d = nc.dram_tensor("wmf", [128, WM_COLS], F32, kind="ExternalInput").ap()
    bias_d = nc.dram_tensor("bias", [128, 16], F32, kind="ExternalInput").ap()
    out_d = nc.dram_tensor("out", [BPC, 9], F32, kind="ExternalOutput").ap()
    with tile.TileContext(nc) as tc:
        _emit_core_program(tc, nc, pt_d, wpos_d, wm16_d, wmf_d, bias_d, out_d,
                           reps=reps)
    nc.compile()
    return nc


def prepare_host_inputs(inputs):
    """Shared (weights) and per-core (points) host-side packing."""
    import ml_dtypes
    BF = ml_dtypes.bfloat16
    f = lambda k: np.ascontiguousarray(np.asarray(inputs[k], np.float32))
    p = f("p")
    W_pos, b_pos = f("W_pos"), f("b_pos")
    W0, b0 = f("W0"), f("b0")
    W1, b1 = f("W1"), f("b1")
    W2, b2 = f("W2"), f("b2")
    W3, b3 = f("W3"), f("b3")
    Wc, bc = f("Wc"), f("bc")
    Wm0, bm0 = f("Wm0"), f("bm0")
    Wm1, bm1 = f("Wm1"), f("bm1")
    Wm2, bm2 = f("Wm2"), f("bm2")
    Wp, bp = f("Wp"), f("bp")

    wpos = np.broadcast_to(W_pos, (NQ, 3, 256)).copy()  # replicated per quad

    wm = np.zeros((128, WM_COLS), np.float32)
    blocks = [W0[:128], W0[128:], W1[:128], W1[128:], W2[:128], W2[128:],
              W3[:128], W3[128:], Wc, Wm0, Wm1, Wm2]
    for i, blk in enumerate(blocks):
        wm[:, 128 * i:128 * (i + 1)] = blk
    wm[:, 1536:1536 + 9] = Wp

    bias = np.zeros((128, 16), np.float32)
    bias[:, BC_BPOS_A] = b_pos[:128]
    bias[:, BC_BPOS_B] = b_pos[128:]
    bias[:, BC_B0] = b0
    bias[:, BC_B1] = b1
    bias[:, BC_B2] = b2
    bias[:, BC_B3] = b3
    bias[:, BC_BC] = bc
    bias[:, BC_BM0] = bm0
    bias[:, BC_BM1] = bm1
    bias[:, BC_BM2] = bm2
    bias[:9, BC_BP] = bp
    bias[:, BC_NEG_B0] = -b0

    shared = {"wpos": wpos.astype(BF), "wm16": wm.astype(BF),
              "wmf": wm, "bias": bias}

    in_maps = []
    for core in range(NCORES):
        pc = p[core * BPC:(core + 1) * BPC]          # [BPC, T, 3]
        pt = np.empty((BPC, NQ, 3, QT), np.float32)
        for b in range(BPC):
            for q in range(NQ):
                pt[b, q] = pc[b, q * QT:(q + 1) * QT, :].T
        in_maps.append({"pt": pt.astype(BF), **shared})
    return in_maps


_PROGRAM_CACHE = {}


def kernel(**inputs):
    reps = 1
    if reps not in _PROGRAM_CACHE:
        _PROGRAM_CACHE[reps] = build_program(reps)
    nc = _PROGRAM_CACHE[reps]
    in_maps = prepare_host_inputs(inputs)
    res = run_bass_kernel_spmd(nc, in_maps, core_ids=list(range(NCORES)))
    out = np.concatenate([res.results[i]["out"] for i in range(NCORES)], axis=0)
    return out.astype(np.float32)


# revision 12
# speedup vs baseline: 1.2627x; 1.0336x over previous
"""FCPlanenet Trainium2 kernel (8-core data-parallel over batch).

Network (per batch of T=8192 points, feature-major [feat, T] on chip):
  net0 = p @ Wpos + bpos            [256, T]   (K=3 matmul, quad-packed)
  net1 = relu(net0) @ W0 + b0       [128, T]   (K=256)
  netk+1 = relu(cat(netk, max_t netk)) @ Wk + bk   for W1..W3
  out = MLP head over max_t net4    [9] per batch

The pooled-concat half is rank-1 (same vector at every point), so each layer
reduces to Wk_a.T @ relu(netk) plus a per-feature constant vector C applied
at drain time.  Big matmuls run in bf16 (weights + activations; ~1e-3 final
rel err, gate is 2e-2).  PSUM-touching elementwise work is split between two
engines (gpsimd cannot access PSUM):
  - ACT:  true-relu drains  r = relu(psum + C_R)          (R-class chunks)
          plus identity drains of some D-stage psums to bf16 scratch
  - DVE:  tensor_scalar+accum drains  r~ = max(psum, -C_T)  (= relu - C_T)
          emitting the per-chunk pooled max in the same pass (T-class)
Pooling coverage of ACT-drained bf16 data costs almost nothing: one DVE
tensor_scalar+accum in 4x mode (0.26 ns/elem, all-SBUF 2-byte).  gpsimd takes
the tiny [128,1] boundary constant ops.  Offset-class constants fold into the
next layer's constants via tiny matvecs (Wa @ C_T, off the critical path).
"""

import os
import sys

import numpy as np

for _p in ("/opt/trn_rl_repo", "/root/.axon_site/_ro/trn_rl_repo"):
    if os.path.isdir(_p) and _p not in sys.path:
        sys.path.insert(0, _p)

import concourse.bass as bass  # noqa: E402
import concourse.tile as tile  # noqa: E402
from concourse import bacc, mybir  # noqa: E402
from concourse.bass_utils import run_bass_kernel_spmd  # noqa: E402

F32 = mybir.dt.float32
F32R = mybir.dt.float32r
BF16 = mybir.dt.bfloat16
AX = mybir.AxisListType.X
MAX = mybir.AluOpType.max
ADD = mybir.AluOpType.add
MUL = mybir.AluOpType.mult
RELU = mybir.ActivationFunctionType.Relu
IDENT = mybir.ActivationFunctionType.Identity

NCORES = 8
B = 32
T = 8192
BPC = B // NCORES  # batches per core
NQ = 4             # point quads (for K=3 matmul row-packing)
QT = T // NQ       # 2048 points per quad
NCH = 512          # matmul free-dim chunk (one PSUM bank)
NSUP = 1024        # drain supertile (2 chunks)
NST = T // NSUP    # 8 supertiles per layer

NEG_INF = -1.0e30

# bias tile columns
BC_BPOS_A, BC_BPOS_B = 0, 1
BC_B0, BC_B1, BC_B2, BC_B3 = 2, 3, 4, 5
BC_BC, BC_BM0, BC_BM1, BC_BM2, BC_BP = 6, 7, 8, 9, 10
BC_NEG_B0 = 11
BC_NEG_B1, BC_NEG_B2, BC_NEG_B3 = 12, 13, 14

# wm tile blocks of 128 cols: w0a w0b w1a w1b w2a w2b w3a w3b wc wm0 wm1 wm2 wp
WM_COLS = 13 * 128 + 16

# ---- engine assignment tables (tuning knobs) ----
# L-layer supertiles 0..7: 'A' = ACT true-relu (R-class, bf16 out, covered by
# one DVE 4x accum pass), 'V' = DVE ts+accum offset form (T-class).
# R-sts sit at stride 3 so ACT and DVE drain work interleaves in time and
# the R-chunk 4x cov pass can still use one regular strided AP.
L_ENG = ("A", "V", "V", "A", "V", "V", "A", "V")
L_RSTS = (0, 3, 6)           # R-class sts (stride 3), cov -> acc col 0
L_ACC_T = {1: 1, 2: 2, 4: 3, 5: 4, 7: 5}
L_NT = 5  # number of T accum cols (starting at col 1)

# pos supertile tasks idx=8*qp+2*s+h: 'A' = ACT, 'V' = DVE (true relu both).
# Batch 0 has no other work to overlap, so its pos spreads onto DVE too.
POS_ENG = ("A",) * 16
POS_ENG_PRO = tuple("V" if i in (2, 5, 8, 11, 13, 15) else "A"
                    for i in range(16))

# D stage: sts in D_ACT drain via ACT-ident to bf16 scratch (one DVE 4x
# accum covers them); the rest are DVE ts+accum psum singles.
# r3 chunk classes follow L_RSTS: sts 0,3,6 are R (C_R^D), rest T (C_T^D).
D_ACT = (4, 5)               # must be same-class (T) and contiguous
D_COL = {0: 0, 3: 1, 6: 2, 1: 3, 2: 4, "act": 5, 7: 6}
D_RCOL = (0, 3)              # accD cols [0,3) are R-class raw maxes
D_TCOL = (3, 7)              # accD cols [3,7) are T-class raw maxes


def _f32r(ap):
    return ap if ap.dtype == F32R else ap.bitcast(F32R)


def _f32(ap):
    return ap if ap.dtype == F32 else ap.bitcast(F32)


def _emit_core_program(tc, nc, pt_d, wpos_d, wm16_d, wmf_d, wmn_d, bias_d,
                       out_d, reps=1):
    ctx_pools = []

    def pool(name, bufs, space="SBUF"):
        p = tc.alloc_tile_pool(name=name, bufs=bufs, space=space)
        ctx_pools.append(p)
        return p

    const = pool("const", 1)
    ptp = pool("ptp", 2)
    r0p = pool("r0p", 1)
    netp = pool("netp", 2)
    smallp = pool("smallp", 8)
    vecp = pool("vecp", 30)
    covp = pool("covp", 2)
    dscp = pool("dscp", 2)
    headp = pool("headp", 2)
    psmm = pool("psmm", 3, space="PSUM")
    psvp = pool("psvp", 2, space="PSUM")

    # ---- constants ----
    wpos_sb = const.tile([99, 256], BF16, name="wpos_sb")
    for q in range(NQ):
        nc.sync.dma_start(wpos_sb[32 * q:32 * q + 3, :], wpos_d[q])
    bias_sb = const.tile([128, 16], F32, name="bias_sb")
    nc.sync.dma_start(bias_sb[:], bias_d)
    wm_sb = const.tile([128, WM_COLS], BF16, name="wm_sb")
    wmf_sb = const.tile([128, WM_COLS], F32, name="wmf_sb")
    wmn_sb = const.tile([128, 384], F32, name="wmn_sb")
    _wm_loaded = [False]

    def load_wm():
        if not _wm_loaded[0]:
            # W0 blocks first: L0 matmuls need them ~1us in; the f32 matvec
            # copy is only needed at the first boundary.
            nc.sync.dma_start(wm_sb[:, 0:256], wm16_d[:, 0:256])
            nc.sync.dma_start(wm_sb[:, 256:WM_COLS], wm16_d[:, 256:WM_COLS])
            nc.sync.dma_start(wmn_sb[:, 0:384], wmn_d[:, 0:384])
            nc.sync.dma_start(wmf_sb[:, 0:WM_COLS], wmf_d[:, 0:WM_COLS])
            _wm_loaded[0] = True

    def W(i):       # bf16 weights for the big matmuls
        return wm_sb[:, 128 * i:128 * (i + 1)]

    def Wf(i):      # f32 weights for [128,1] matvecs
        return wmf_sb[:, 128 * i:128 * (i + 1)]

    def negWb(li):  # f32 -W1b/-W2b/-W3b for the negated boundary chain
        return wmn_sb[:, 128 * li:128 * (li + 1)]

    def bcol(i):
        return bias_sb[:, i:i + 1]

    def mk_acc(name):
        """Accum tile, initialized to -inf: the HW tensor_scalar accum_out
        read-modify-writes the destination."""
        acc = smallp.tile([128, 8], F32, tag="pp", name=name)
        nc.gpsimd.memset(acc[:], NEG_INF)
        return acc

    def pos_tasks(b, pt_sb, r0):
        """16 supertile tasks for the pos layer of batch b (true relu)."""
        eng = POS_ENG_PRO if b == 0 else POS_ENG
        tasks = []
        for qp in range(2):
            for s in range(4):
                for h in range(2):
                    def t(qp=qp, s=s, h=h):
                        ps = psmm.tile([128, NSUP], F32, tag="mm", name="ps_pos")
                        for jq in range(2):
                            q = 2 * qp + jq
                            nc.tensor.matmul(
                                ps[:, NCH * jq:NCH * (jq + 1)],
                                wpos_sb[32 * q:32 * q + 3, 128 * h:128 * (h + 1)],
                                pt_sb[32 * q:32 * q + 3, NCH * s:NCH * (s + 1)],
                                start=True, stop=True,
                                tile_position=(32 * q, 0),
                            )
                        g0 = 8 * qp + s
                        dst = (r0[h].rearrange("p (g c) -> p g c", c=NCH)
                               [:, g0:g0 + 5:4, :])
                        srcv = ps.rearrange("p (g c) -> p g c", c=NCH)
                        idx = 8 * qp + 2 * s + h
                        if eng[idx] == "V":
                            nc.vector.tensor_scalar(dst, srcv, bcol(BC_BPOS_A + h),
                                                    0.0, op0=ADD, op1=MAX)
                        else:
                            nc.scalar.activation(dst, srcv, RELU,
                                                 bias=bcol(BC_BPOS_A + h))
                    tasks.append(t)
        return tasks

    def layer_tasks(li, b, r0, r_prev, r_out, acc, consts):
        """Supertile tasks for pooled layer li (0..2).  consts = (cR, cT, negT)
        access thunks (bound at emit time).  acc: [128,8] f32 accum tile.
        After the last R drain one DVE 4x pass covers their pooling max."""
        cR, cT, negT = consts
        tasks = []

        def emit_st(st, li):
            ps = psmm.tile([128, NSUP], F32, tag="mm", name=f"ps_l{li}")
            for j in range(2):
                c = 2 * st + j
                osl = ps[:, NCH * j:NCH * (j + 1)]
                csl = slice(NCH * c, NCH * (c + 1))
                if li == 0:
                    nc.tensor.matmul(osl, W(0), r0[0][:, csl],
                                     start=True, stop=False)
                    nc.tensor.matmul(osl, W(1), r0[1][:, csl],
                                     start=False, stop=True)
                else:
                    nc.tensor.matmul(osl, W(2 * li), r_prev[:, csl],
                                     start=True, stop=True)
            dsl = slice(NSUP * st, NSUP * (st + 1))
            if L_ENG[st] == "A":
                nc.scalar.activation(r_out[:, dsl], ps[:], RELU, bias=cR())
                if st == L_RSTS[-1]:
                    cov = covp.tile([128, len(L_RSTS) * NSUP], BF16, tag="cov",
                                    name="cov")
                    rsrc = (r_out.rearrange("p (g c) -> p g c", c=NSUP)
                            [:, L_RSTS[0]:L_RSTS[-1] + 1:3, :])
                    nc.vector.tensor_scalar(
                        cov.rearrange("p (g c) -> p g c", c=NSUP), rsrc,
                        NEG_INF, NEG_INF, op0=MAX, op1=MAX,
                        accum_out=acc[:, 0:1])
            else:
                col = L_ACC_T[st]
                nc.vector.tensor_scalar(r_out[:, dsl], ps[:], negT(), NEG_INF,
                                        op0=MAX, op1=MAX,
                                        accum_out=acc[:, col:col + 1])

        for st in range(NST):
            tasks.append(lambda st=st, li=li: emit_st(st, li))
        return tasks

    def d_tasks(b, r_prev, accD, last=False):
        """D-stage supertiles: matmuls + raw psum maxes into accD.
        Returns (rcol_range, tcol_range) for the final reduces.  The last
        batch has no pos fillers for ACT, so it drains everything via
        ACT-ident + two DVE 4x covs instead of DVE psum singles."""
        tasks = []
        if last:
            groups = {"T": (1, 2, 4, 5, 7), "R": (0, 3, 6)}
            gcol = {"R": 0, "T": 1}
            singles = {}
            ranges = ((0, 1), (1, 2))
        else:
            groups = {"T": D_ACT}
            gcol = {"T": D_COL["act"]}
            singles = {st: D_COL[st] for st in range(NST)
                       if st not in D_ACT}
            ranges = (D_RCOL, D_TCOL)
        dscr = {g: dscp.tile([128, len(sts) * NSUP], BF16, tag=f"dsc{g}",
                             name=f"dscr{g}_{b}")
                for g, sts in groups.items()}
        member = {st: (g, k) for g, sts in groups.items()
                  for k, st in enumerate(sts)}

        def emit_st(st):
            ps = psmm.tile([128, NSUP], F32, tag="mm", name="ps_d")
            for j in range(2):
                c = 2 * st + j
                csl = slice(NCH * c, NCH * (c + 1))
                nc.tensor.matmul(ps[:, NCH * j:NCH * (j + 1)], W(6),
                                 r_prev[:, csl], start=True, stop=True)
            if st in member:
                g, k = member[st]
                nc.scalar.activation(dscr[g][:, NSUP * k:NSUP * (k + 1)],
                                     ps[:], IDENT, bias=0.0)
                if st == groups[g][-1]:
                    cov = covp.tile([128, len(groups[g]) * NSUP], BF16,
                                    tag="cov", name=f"covd{g}")
                    col = gcol[g]
                    nc.vector.tensor_scalar(cov[:], dscr[g][:], NEG_INF,
                                            NEG_INF, op0=MAX, op1=MAX,
                                            accum_out=accD[:, col:col + 1])
            else:
                col = singles[st]
                scrj = headp.tile([128, NSUP], BF16, tag="scrj", name="scrj")
                nc.vector.tensor_scalar(scrj[:], ps[:], NEG_INF, NEG_INF,
                                        op0=MAX, op1=MAX,
                                        accum_out=accD[:, col:col + 1])

        for st in range(NST):
            tasks.append(lambda st=st: emit_st(st))
        return tasks, ranges

    def interleave(a, bl):
        out = []
        n = max(len(a), len(bl))
        for i in range(n):
            if i < len(a):
                out.append(a[i])
            if i < len(bl):
                out.append(bl[i])
        return out

    import contextlib

    def _rep_scope():
        if reps > 1:
            return tc.For_i(0, reps, 1,
                            hint_engines=(mybir.EngineType.PE,
                                          mybir.EngineType.Activation,
                                          mybir.EngineType.DVE,
                                          mybir.EngineType.Pool))
        return contextlib.nullcontext()

    with _rep_scope():
        # per-batch state created lazily
        def new_batch_state(b):
            pt_sb = ptp.tile([99, QT], BF16, tag="pt", name="pt_sb")
            for q in range(NQ):
                nc.sync.dma_start(pt_sb[32 * q:32 * q + 3, :], pt_d[b, q])
            r0a = r0p.tile([128, T], BF16, tag="r0a", name="r0a")
            r0b = r0p.tile([128, T], BF16, tag="r0b", name="r0b")
            return pt_sb, (r0a, r0b)

        def mk_consts_L0():
            # C_R = C_T = b0 (pos chunks are all true-relu)
            return (lambda: bcol(BC_B0), lambda: bcol(BC_B0),
                    lambda: bcol(BC_NEG_B0))

        def boundary(li, b, acc, cur):
            """After layer li (0..2): compute m and next-layer constants from
            the finished layer's consts `cur` = (cR, cT, negT) thunks.
            Critical chain is 3 sem hops: reduce+stt (DVE) -> psvN (PE) ->
            negT stt (DVE).  acc col 0 = max_t relu(net) over R-chunks
            (final form); cols 1..L_NT = max(max_t psum, -C_T) (T-chunks)."""
            wa_i = 2 * (li + 1)
            bc_i = BC_B1 + li
            nbc_i = BC_NEG_B1 + li
            cT_cur, negT_cur = cur[1](), cur[2]()
            # pooled max m = max(reduce(accT) + C_T, accR)
            mT = vecp.tile([128, 1], F32, tag="v", name=f"mT{li}_{b}")
            nc.vector.tensor_reduce(mT, acc[:, 1:1 + L_NT], AX, MAX)
            m = vecp.tile([128, 1], F32, tag="v", name=f"m{li}_{b}")
            nc.vector.scalar_tensor_tensor(m, mT, cT_cur, acc[:, 0:1],
                                           op0=ADD, op1=MAX)
            # psv2N = Wa @ (-C_T) (issued early, off the m critical path)
            psv2 = psvp.tile([128, 1], F32, tag="psv", name=f"psv2_{li}_{b}")
            nc.tensor.matmul(psv2[:], Wf(wa_i), negT_cur, start=True, stop=True)
            psv2s = vecp.tile([128, 1], F32, tag="v", name=f"p2s{li}_{b}")
            nc.scalar.activation(psv2s, psv2[:], IDENT, bias=0.0)
            # psvN = (-Wb) @ m
            psv = psvp.tile([128, 1], F32, tag="psv", name=f"psv_{li}_{b}")
            nc.tensor.matmul(psv[:], negWb(li), m, start=True, stop=True)
            # negT' = (psvN + (-b)) + psv2N  = -(Wb m + b + Wa C_T)
            negT = vecp.tile([128, 1], F32, tag="v", name=f"nT{li}_{b}")
            nc.vector.scalar_tensor_tensor(negT, psv[:], bcol(nbc_i), psv2s,
                                           op0=ADD, op1=ADD)
            # positive forms, off the critical path
            cR = vecp.tile([128, 1], F32, tag="v", name=f"cR{li}_{b}")
            nc.scalar.activation(cR, psv[:], IDENT, bias=bcol(bc_i), scale=-1.0)
            cT = vecp.tile([128, 1], F32, tag="v", name=f"cT{li}_{b}")
            nc.gpsimd.tensor_scalar(cT, negT, -1.0, 0.0, op0=MUL, op1=ADD)
            consts = (lambda: cR, lambda: cT, lambda: negT)
            return consts, cT

        # prologue: pos(0) interleaved with L0(0) in chunk-ready order
        st0 = new_batch_state(0)
        load_wm()
        states = {0: st0}
        p0 = pos_tasks(0, st0[0], st0[1])
        consts0 = mk_consts_L0()
        r1_0 = netp.tile([128, T], BF16, tag="net", name="r1_0")
        acc0_0 = mk_acc("a0_0")
        l0_0 = layer_tasks(0, 0, st0[1], None, r1_0, acc0_0, consts0)
        for t in p0[0:4]:
            t()
        l0_0[0](); l0_0[2]()
        for t in p0[4:8]:
            t()
        l0_0[1](); l0_0[3]()
        for t in p0[8:12]:
            t()
        l0_0[4](); l0_0[6]()
        for t in p0[12:16]:
            t()
        l0_0[5](); l0_0[7]()

        for b in range(BPC):
            _, r0 = states[b]

            if b == 0:
                r1, acc0 = r1_0, acc0_0
            else:
                consts0 = mk_consts_L0()
                r1 = netp.tile([128, T], BF16, tag="net", name=f"r1_{b}")
                acc0 = mk_acc(f"a0_{b}")
                for t in layer_tasks(0, b, r0, None, r1, acc0, consts0):
                    t()

            filler = []
            if b + 1 < BPC:
                stn = new_batch_state(b + 1)
                states[b + 1] = stn
                filler = pos_tasks(b + 1, stn[0], stn[1])
            for t in filler[0:2]:
                t()
            consts1, cT1 = boundary(0, b, acc0, consts0)

            r2 = netp.tile([128, T], BF16, tag="net", name=f"r2_{b}")
            acc1 = mk_acc(f"a1_{b}")
            for t in interleave(layer_tasks(1, b, None, r1, r2, acc1, consts1),
                                filler[2:5]):
                t()
            consts2, cT2 = boundary(1, b, acc1, consts1)

            r3 = netp.tile([128, T], BF16, tag="net", name=f"r3_{b}")
            acc2 = mk_acc(f"a2_{b}")
            for t in interleave(layer_tasks(2, b, None, r2, r3, acc2, consts2),
                                filler[5:10]):
                t()
            constsD, cTD = boundary(2, b, acc2, consts2)
            cRD = constsD[0]()

            accD = mk_acc(f"aD_{b}")
            dts, (rcols, tcols) = d_tasks(b, r3, accD, last=(b + 1 == BPC))
            for t in interleave(dts, filler[10:16]):
                t()

            # s = relu(max(reduce(accD_T) + C_T^D, reduce(accD_R) + C_R^D))
            # (all DVE: engine-internal ordering, no cross-engine sem hops)
            sR0 = vecp.tile([128, 1], F32, tag="v", name=f"sR0_{b}")
            nc.vector.tensor_reduce(sR0, accD[:, rcols[0]:rcols[1]], AX, MAX)
            sRc = vecp.tile([128, 1], F32, tag="v", name=f"sRc_{b}")
            nc.vector.tensor_scalar(sRc, sR0, cRD, 0.0, op0=ADD, op1=ADD)
            sT0 = vecp.tile([128, 1], F32, tag="v", name=f"sT0_{b}")
            nc.vector.tensor_reduce(sT0, accD[:, tcols[0]:tcols[1]], AX, MAX)
            spre = vecp.tile([128, 1], F32, tag="v", name=f"sp_{b}")
            nc.vector.scalar_tensor_tensor(spre, sT0, cTD, sRc,
                                           op0=ADD, op1=MAX)
            s_b = vecp.tile([128, 1], F32, tag="v", name=f"s_{b}")
            nc.vector.tensor_scalar(s_b, spre, 0.0, 0.0, op0=MAX, op1=ADD)

            # incremental head for this batch (all tiny fp32 ops)
            hb = s_b
            for wi, bi in ((8, BC_BC), (9, BC_BM0), (10, BC_BM1), (11, BC_BM2)):
                ps = psvp.tile([128, 1], F32, tag="psv", name=f"ph{wi}_{b}")
                nc.tensor.matmul(ps[:], Wf(wi), hb[:], start=True, stop=True)
                h2 = vecp.tile([128, 1], F32, tag="v", name=f"h{wi}_{b}")
                nc.scalar.activation(h2, ps[:], RELU, bias=bcol(bi))
                hb = h2
            ps9 = psvp.tile([9, 1], F32, tag="psv", name=f"po_{b}")
            nc.tensor.matmul(ps9[:], wmf_sb[:, 1536:1536 + 9], hb[:],
                             start=True, stop=True)
            ob = headp.tile([9, 1], F32, tag="o", name=f"ob_{b}")
            nc.scalar.activation(ob, ps9[:], IDENT, bias=bias_sb[0:9, BC_BP:BC_BP + 1])
            nc.sync.dma_start(out_d[b:b + 1].rearrange("b f -> f b"), ob[:])

    for p in reversed(ctx_pools):
        p.release()


def build_program(reps=1):
    nc = bacc.Bacc("TRN2", target_bir_lowering=False, debug=False,
                   num_devices=NCORES)
    pt_d = nc.dram_tensor("pt", [BPC, NQ, 3, QT], BF16, kind="ExternalInput").ap()
    wpos_d = nc.dram_tensor("wpos", [NQ, 3, 256], BF16, kind="ExternalInput").ap()
    wm16_d = nc.dram_tensor("wm16", [128, WM_COLS], BF16, kind="ExternalInput").ap()
    wmf_d = nc.dram_tensor("wmf", [128, WM_COLS], F32, kind="ExternalInput").ap()
    wmn_d = nc.dram_tensor("wmn", [128, 384], F32, kind="ExternalInput").ap()
    bias_d = nc.dram_tensor("bias", [128, 16], F32, kind="ExternalInput").ap()
    out_d = nc.dram_tensor("out", [BPC, 9], F32, kind="ExternalOutput").ap()
    with tile.TileContext(nc) as tc:
        _emit_core_program(tc, nc, pt_d, wpos_d, wm16_d, wmf_d, wmn_d,
                           bias_d, out_d, reps=reps)
    nc.compile()
    return nc


def prepare_host_inputs(inputs):
    """Shared (weights) and per-core (points) host-side packing."""
    import ml_dtypes
    BF = ml_dtypes.bfloat16
    f = lambda k: np.ascontiguousarray(np.asarray(inputs[k], np.float32))
    p = f("p")
    W_pos, b_pos = f("W_pos"), f("b_pos")
    W0, b0 = f("W0"), f("b0")
    W1, b1 = f("W1"), f("b1")
    W2, b2 = f("W2"), f("b2")
    W3, b3 = f("W3"), f("b3")
    Wc, bc = f("Wc"), f("bc")
    Wm0, bm0 = f("Wm0"), f("bm0")
    Wm1, bm1 = f("Wm1"), f("bm1")
    Wm2, bm2 = f("Wm2"), f("bm2")
    Wp, bp = f("Wp"), f("bp")

    wpos = np.broadcast_to(W_pos, (NQ, 3, 256)).copy()  # replicated per quad

    wm = np.zeros((128, WM_COLS), np.float32)
    blocks = [W0[:128], W0[128:], W1[:128], W1[128:], W2[:128], W2[128:],
              W3[:128], W3[128:], Wc, Wm0, Wm1, Wm2]
    for i, blk in enumerate(blocks):
        wm[:, 128 * i:128 * (i + 1)] = blk
    wm[:, 1536:1536 + 9] = Wp

    bias = np.zeros((128, 16), np.float32)
    bias[:, BC_BPOS_A] = b_pos[:128]
    bias[:, BC_BPOS_B] = b_pos[128:]
    bias[:, BC_B0] = b0
    bias[:, BC_B1] = b1
    bias[:, BC_B2] = b2
    bias[:, BC_B3] = b3
    bias[:, BC_BC] = bc
    bias[:, BC_BM0] = bm0
    bias[:, BC_BM1] = bm1
    bias[:, BC_BM2] = bm2
    bias[:9, BC_BP] = bp
    bias[:, BC_NEG_B0] = -b0
    bias[:, BC_NEG_B1] = -b1
    bias[:, BC_NEG_B2] = -b2
    bias[:, BC_NEG_B3] = -b3

    wmn = -np.concatenate([W1[128:], W2[128:], W3[128:]], axis=1)
    shared = {"wpos": wpos.astype(BF), "wm16": wm.astype(BF),
              "wmf": wm, "wmn": np.ascontiguousarray(wmn), "bias": bias}

    in_maps = []
    for core in range(NCORES):
        pc = p[core * BPC:(core + 1) * BPC]          # [BPC, T, 3]
        pt = np.empty((BPC, NQ, 3, QT), np.float32)
        for b in range(BPC):
            for q in range(NQ):
                pt[b, q] = pc[b, q * QT:(q + 1) * QT, :].T
        in_maps.append({"pt": pt.astype(BF), **shared})
    return in_maps


_PROGRAM_CACHE = {}


def kernel(**inputs):
    reps = 1
    if reps not in _PROGRAM_CACHE:
        _PROGRAM_CACHE[reps] = build_program(reps)
    nc = _PROGRAM_CACHE[reps]
    in_maps = prepare_host_inputs(inputs)
    res = run_bass_kernel_spmd(nc, in_maps, core_ids=list(range(NCORES)))
    out = np.concatenate([res.results[i]["out"] for i in range(NCORES)], axis=0)
    return out.astype(np.float32)


# revision 13
# speedup vs baseline: 1.2628x; 1.0001x over previous
"""FCPlanenet Trainium2 kernel (8-core data-parallel over batch).

Network (per batch of T=8192 points, feature-major [feat, T] on chip):
  net0 = p @ Wpos + bpos            [256, T]   (K=3 matmul, quad-packed)
  net1 = relu(net0) @ W0 + b0       [128, T]   (K=256)
  netk+1 = relu(cat(netk, max_t netk)) @ Wk + bk   for W1..W3
  out = MLP head over max_t net4    [9] per batch

The pooled-concat half is rank-1 (same vector at every point), so each layer
reduces to Wk_a.T @ relu(netk) plus a per-feature constant vector C applied
at drain time.  Big matmuls run in bf16 (weights + activations; ~1e-3 final
rel err, gate is 2e-2).  PSUM-touching elementwise work is split between two
engines (gpsimd cannot access PSUM):
  - ACT:  true-relu drains  r = relu(psum + C_R)          (R-class chunks)
          plus identity drains of some D-stage psums to bf16 scratch
  - DVE:  tensor_scalar+accum drains  r~ = max(psum, -C_T)  (= relu - C_T)
          emitting the per-chunk pooled max in the same pass (T-class)
Pooling coverage of ACT-drained bf16 data costs almost nothing: one DVE
tensor_scalar+accum in 4x mode (0.26 ns/elem, all-SBUF 2-byte).  gpsimd takes
the tiny [128,1] boundary constant ops.  Offset-class constants fold into the
next layer's constants via tiny matvecs (Wa @ C_T, off the critical path).
"""

import os
import sys

import numpy as np

for _p in ("/opt/trn_rl_repo", "/root/.axon_site/_ro/trn_rl_repo"):
    if os.path.isdir(_p) and _p not in sys.path:
        sys.path.insert(0, _p)

import concourse.bass as bass  # noqa: E402
import concourse.tile as tile  # noqa: E402
from concourse import bacc, mybir  # noqa: E402
from concourse.bass_utils import run_bass_kernel_spmd  # noqa: E402

F32 = mybir.dt.float32
F32R = mybir.dt.float32r
BF16 = mybir.dt.bfloat16
AX = mybir.AxisListType.X
MAX = mybir.AluOpType.max
ADD = mybir.AluOpType.add
MUL = mybir.AluOpType.mult
RELU = mybir.ActivationFunctionType.Relu
IDENT = mybir.ActivationFunctionType.Identity

NCORES = 8
B = 32
T = 8192
BPC = B // NCORES  # batches per core
NQ = 4             # point quads (for K=3 matmul row-packing)
QT = T // NQ       # 2048 points per quad
NCH = 512          # matmul free-dim chunk (one PSUM bank)
NSUP = 1024        # drain supertile (2 chunks)
NST = T // NSUP    # 8 supertiles per layer

NEG_INF = -1.0e30

# bias tile columns
BC_BPOS_A, BC_BPOS_B = 0, 1
BC_B0, BC_B1, BC_B2, BC_B3 = 2, 3, 4, 5
BC_BC, BC_BM0, BC_BM1, BC_BM2, BC_BP = 6, 7, 8, 9, 10
BC_NEG_B0 = 11
BC_NEG_B1, BC_NEG_B2, BC_NEG_B3 = 12, 13, 14

# wm tile blocks of 128 cols: w0a w0b w1a w1b w2a w2b w3a w3b wc wm0 wm1 wm2 wp
WM_COLS = 13 * 128 + 16

# ---- engine assignment tables (tuning knobs) ----
# L-layer supertiles 0..7: 'A' = ACT true-relu (R-class, bf16 out, covered by
# one DVE 4x accum pass), 'V' = DVE ts+accum offset form (T-class).
# R-sts sit at stride 3 so ACT and DVE drain work interleaves in time and
# the R-chunk 4x cov pass can still use one regular strided AP.
L_ENG = ("A", "V", "V", "A", "V", "V", "A", "V")
L_RSTS = (0, 3, 6)           # R-class sts (stride 3), cov -> acc col 0
L_ACC_T = {1: 1, 2: 2, 4: 3, 5: 4, 7: 5}
L_NT = 5  # number of T accum cols (starting at col 1)

# pos supertile tasks idx=8*qp+2*s+h: 'A' = ACT, 'V' = DVE (true relu both).
# Batch 0 has no other work to overlap, so its pos spreads onto DVE too.
POS_ENG = ("A",) * 16
POS_ENG_PRO = tuple("V" if i in (2, 5, 8, 11, 13, 15) else "A"
                    for i in range(16))

# D stage: sts in D_ACT drain via ACT-ident to bf16 scratch (one DVE 4x
# accum covers them); the rest are DVE ts+accum psum singles.
# r3 chunk classes follow L_RSTS: sts 0,3,6 are R (C_R^D), rest T (C_T^D).
D_ACT = (4, 5)               # must be same-class (T) and contiguous
D_COL = {0: 0, 3: 1, 6: 2, 1: 3, 2: 4, "act": 5, 7: 6}
D_RCOL = (0, 3)              # accD cols [0,3) are R-class raw maxes
D_TCOL = (3, 7)              # accD cols [3,7) are T-class raw maxes


def _f32r(ap):
    return ap if ap.dtype == F32R else ap.bitcast(F32R)


def _f32(ap):
    return ap if ap.dtype == F32 else ap.bitcast(F32)


def _emit_core_program(tc, nc, pt_d, wpos_d, wm16_d, wmf_d, wmn_d, bias_d,
                       out_d, reps=1):
    ctx_pools = []

    def pool(name, bufs, space="SBUF"):
        p = tc.alloc_tile_pool(name=name, bufs=bufs, space=space)
        ctx_pools.append(p)
        return p

    const = pool("const", 1)
    ptp = pool("ptp", 2)
    r0p = pool("r0p", 1)
    netp = pool("netp", 2)
    smallp = pool("smallp", 8)
    vecp = pool("vecp", 30)
    covp = pool("covp", 2)
    dscp = pool("dscp", 2)
    headp = pool("headp", 2)
    psmm = pool("psmm", 3, space="PSUM")
    psvp = pool("psvp", 2, space="PSUM")

    # ---- constants ----
    bias_sb = const.tile([128, 16], F32, name="bias_sb")
    nc.sync.dma_start(bias_sb[:], bias_d)
    wpos_sb = const.tile([99, 256], BF16, name="wpos_sb")
    for q in range(2):
        nc.sync.dma_start(wpos_sb[32 * q:32 * q + 3, :], wpos_d[q])
    _wpos_rest = [False]

    def load_wpos_rest():
        if not _wpos_rest[0]:
            for q in range(2, NQ):
                nc.sync.dma_start(wpos_sb[32 * q:32 * q + 3, :], wpos_d[q])
            _wpos_rest[0] = True
    wm_sb = const.tile([128, WM_COLS], BF16, name="wm_sb")
    wmf_sb = const.tile([128, WM_COLS], F32, name="wmf_sb")
    wmn_sb = const.tile([128, 384], F32, name="wmn_sb")
    _wm_loaded = [False]

    def load_wm():
        if not _wm_loaded[0]:
            # W0 blocks first: L0 matmuls need them ~1us in; the f32 matvec
            # copy is only needed at the first boundary.
            nc.sync.dma_start(wm_sb[:, 0:256], wm16_d[:, 0:256])
            nc.sync.dma_start(wm_sb[:, 256:WM_COLS], wm16_d[:, 256:WM_COLS])
            nc.sync.dma_start(wmn_sb[:, 0:384], wmn_d[:, 0:384])
            nc.sync.dma_start(wmf_sb[:, 0:WM_COLS], wmf_d[:, 0:WM_COLS])
            _wm_loaded[0] = True

    def W(i):       # bf16 weights for the big matmuls
        return wm_sb[:, 128 * i:128 * (i + 1)]

    def Wf(i):      # f32 weights for [128,1] matvecs
        return wmf_sb[:, 128 * i:128 * (i + 1)]

    def negWb(li):  # f32 -W1b/-W2b/-W3b for the negated boundary chain
        return wmn_sb[:, 128 * li:128 * (li + 1)]

    def bcol(i):
        return bias_sb[:, i:i + 1]

    def mk_acc(name):
        """Accum tile, initialized to -inf: the HW tensor_scalar accum_out
        read-modify-writes the destination."""
        acc = smallp.tile([128, 8], F32, tag="pp", name=name)
        nc.gpsimd.memset(acc[:], NEG_INF)
        return acc

    def pos_tasks(b, pt_sb, r0):
        """16 supertile tasks for the pos layer of batch b (true relu)."""
        eng = POS_ENG_PRO if b == 0 else POS_ENG
        tasks = []
        for qp in range(2):
            for s in range(4):
                for h in range(2):
                    def t(qp=qp, s=s, h=h):
                        ps = psmm.tile([128, NSUP], F32, tag="mm", name="ps_pos")
                        for jq in range(2):
                            q = 2 * qp + jq
                            nc.tensor.matmul(
                                ps[:, NCH * jq:NCH * (jq + 1)],
                                wpos_sb[32 * q:32 * q + 3, 128 * h:128 * (h + 1)],
                                pt_sb[32 * q:32 * q + 3, NCH * s:NCH * (s + 1)],
                                start=True, stop=True,
                                tile_position=(32 * q, 0),
                            )
                        g0 = 8 * qp + s
                        dst = (r0[h].rearrange("p (g c) -> p g c", c=NCH)
                               [:, g0:g0 + 5:4, :])
                        srcv = ps.rearrange("p (g c) -> p g c", c=NCH)
                        idx = 8 * qp + 2 * s + h
                        if eng[idx] == "V":
                            nc.vector.tensor_scalar(dst, srcv, bcol(BC_BPOS_A + h),
                                                    0.0, op0=ADD, op1=MAX)
                        else:
                            nc.scalar.activation(dst, srcv, RELU,
                                                 bias=bcol(BC_BPOS_A + h))
                    tasks.append(t)
        return tasks

    def layer_tasks(li, b, r0, r_prev, r_out, acc, consts):
        """Supertile tasks for pooled layer li (0..2).  consts = (cR, cT, negT)
        access thunks (bound at emit time).  acc: [128,8] f32 accum tile.
        After the last R drain one DVE 4x pass covers their pooling max."""
        cR, cT, negT = consts
        tasks = []

        def emit_st(st, li):
            ps = psmm.tile([128, NSUP], F32, tag="mm", name=f"ps_l{li}")
            for j in range(2):
                c = 2 * st + j
                osl = ps[:, NCH * j:NCH * (j + 1)]
                csl = slice(NCH * c, NCH * (c + 1))
                if li == 0:
                    nc.tensor.matmul(osl, W(0), r0[0][:, csl],
                                     start=True, stop=False)
                    nc.tensor.matmul(osl, W(1), r0[1][:, csl],
                                     start=False, stop=True)
                else:
                    nc.tensor.matmul(osl, W(2 * li), r_prev[:, csl],
                                     start=True, stop=True)
            dsl = slice(NSUP * st, NSUP * (st + 1))
            if L_ENG[st] == "A":
                nc.scalar.activation(r_out[:, dsl], ps[:], RELU, bias=cR())
                if st == L_RSTS[-1]:
                    cov = covp.tile([128, len(L_RSTS) * NSUP], BF16, tag="cov",
                                    name="cov")
                    rsrc = (r_out.rearrange("p (g c) -> p g c", c=NSUP)
                            [:, L_RSTS[0]:L_RSTS[-1] + 1:3, :])
                    nc.vector.tensor_scalar(
                        cov.rearrange("p (g c) -> p g c", c=NSUP), rsrc,
                        NEG_INF, NEG_INF, op0=MAX, op1=MAX,
                        accum_out=acc[:, 0:1])
            else:
                col = L_ACC_T[st]
                nc.vector.tensor_scalar(r_out[:, dsl], ps[:], negT(), NEG_INF,
                                        op0=MAX, op1=MAX,
                                        accum_out=acc[:, col:col + 1])

        for st in range(NST):
            tasks.append(lambda st=st, li=li: emit_st(st, li))
        return tasks

    def d_tasks(b, r_prev, accD, last=False):
        """D-stage supertiles: matmuls + raw psum maxes into accD.
        Returns (rcol_range, tcol_range) for the final reduces.  The last
        batch has no pos fillers for ACT, so it drains everything via
        ACT-ident + two DVE 4x covs instead of DVE psum singles."""
        tasks = []
        if last:
            groups = {"T": (1, 2, 4, 5)}
            gcol = {"T": 4}
            singles = {0: 0, 3: 1, 6: 2, 7: 3}
            ranges = ((0, 3), (3, 5))
        else:
            groups = {"T": D_ACT}
            gcol = {"T": D_COL["act"]}
            singles = {st: D_COL[st] for st in range(NST)
                       if st not in D_ACT}
            ranges = (D_RCOL, D_TCOL)
        dscr = {g: dscp.tile([128, len(sts) * NSUP], BF16, tag=f"dsc{g}",
                             name=f"dscr{g}_{b}")
                for g, sts in groups.items()}
        member = {st: (g, k) for g, sts in groups.items()
                  for k, st in enumerate(sts)}

        def emit_st(st):
            ps = psmm.tile([128, NSUP], F32, tag="mm", name="ps_d")
            for j in range(2):
                c = 2 * st + j
                csl = slice(NCH * c, NCH * (c + 1))
                nc.tensor.matmul(ps[:, NCH * j:NCH * (j + 1)], W(6),
                                 r_prev[:, csl], start=True, stop=True)
            if st in member:
                g, k = member[st]
                nc.scalar.activation(dscr[g][:, NSUP * k:NSUP * (k + 1)],
                                     ps[:], IDENT, bias=0.0)
                if st == groups[g][-1]:
                    cov = covp.tile([128, len(groups[g]) * NSUP], BF16,
                                    tag="cov", name=f"covd{g}")
                    col = gcol[g]
                    nc.vector.tensor_scalar(cov[:], dscr[g][:], NEG_INF,
                                            NEG_INF, op0=MAX, op1=MAX,
                                            accum_out=accD[:, col:col + 1])
            else:
                col = singles[st]
                scrj = headp.tile([128, NSUP], BF16, tag="scrj", name="scrj")
                nc.vector.tensor_scalar(scrj[:], ps[:], NEG_INF, NEG_INF,
                                        op0=MAX, op1=MAX,
                                        accum_out=accD[:, col:col + 1])

        for st in range(NST):
            tasks.append(lambda st=st: emit_st(st))
        return tasks, ranges

    def interleave(a, bl):
        out = []
        n = max(len(a), len(bl))
        for i in range(n):
            if i < len(a):
                out.append(a[i])
            if i < len(bl):
                out.append(bl[i])
        return out

    import contextlib

    def _rep_scope():
        if reps > 1:
            return tc.For_i(0, reps, 1,
                            hint_engines=(mybir.EngineType.PE,
                                          mybir.EngineType.Activation,
                                          mybir.EngineType.DVE,
                                          mybir.EngineType.Pool))
        return contextlib.nullcontext()

    with _rep_scope():
        # per-batch state created lazily
        def new_batch_state(b):
            pt_sb = ptp.tile([99, QT], BF16, tag="pt", name="pt_sb")
            for q in range(NQ):
                nc.sync.dma_start(pt_sb[32 * q:32 * q + 3, :], pt_d[b, q])
            r0a = r0p.tile([128, T], BF16, tag="r0a", name="r0a")
            r0b = r0p.tile([128, T], BF16, tag="r0b", name="r0b")
            return pt_sb, (r0a, r0b)

        def mk_consts_L0():
            # C_R = C_T = b0 (pos chunks are all true-relu)
            return (lambda: bcol(BC_B0), lambda: bcol(BC_B0),
                    lambda: bcol(BC_NEG_B0))

        def boundary(li, b, acc, cur):
            """After layer li (0..2): compute m and next-layer constants from
            the finished layer's consts `cur` = (cR, cT, negT) thunks.
            Critical chain is 3 sem hops: reduce+stt (DVE) -> psvN (PE) ->
            negT stt (DVE).  acc col 0 = max_t relu(net) over R-chunks
            (final form); cols 1..L_NT = max(max_t psum, -C_T) (T-chunks)."""
            wa_i = 2 * (li + 1)
            bc_i = BC_B1 + li
            nbc_i = BC_NEG_B1 + li
            cT_cur, negT_cur = cur[1](), cur[2]()
            # pooled max m = max(reduce(accT) + C_T, accR)
            mT = vecp.tile([128, 1], F32, tag="v", name=f"mT{li}_{b}")
            nc.vector.tensor_reduce(mT, acc[:, 1:1 + L_NT], AX, MAX)
            m = vecp.tile([128, 1], F32, tag="v", name=f"m{li}_{b}")
            nc.vector.scalar_tensor_tensor(m, mT, cT_cur, acc[:, 0:1],
                                           op0=ADD, op1=MAX)
            # psv2N = Wa @ (-C_T) (issued early, off the m critical path)
            psv2 = psvp.tile([128, 1], F32, tag="psv", name=f"psv2_{li}_{b}")
            nc.tensor.matmul(psv2[:], Wf(wa_i), negT_cur, start=True, stop=True)
            psv2s = vecp.tile([128, 1], F32, tag="v", name=f"p2s{li}_{b}")
            nc.vector.tensor_scalar(psv2s, psv2[:], 0.0, 0.0, op0=ADD, op1=ADD)
            # psvN = (-Wb) @ m
            psv = psvp.tile([128, 1], F32, tag="psv", name=f"psv_{li}_{b}")
            nc.tensor.matmul(psv[:], negWb(li), m, start=True, stop=True)
            # negT' = (psvN + (-b)) + psv2N  = -(Wb m + b + Wa C_T)
            negT = vecp.tile([128, 1], F32, tag="v", name=f"nT{li}_{b}")
            nc.vector.scalar_tensor_tensor(negT, psv[:], bcol(nbc_i), psv2s,
                                           op0=ADD, op1=ADD)
            # positive forms, off the critical path
            cR = vecp.tile([128, 1], F32, tag="v", name=f"cR{li}_{b}")
            nc.scalar.activation(cR, psv[:], IDENT, bias=bcol(bc_i), scale=-1.0)
            cT = vecp.tile([128, 1], F32, tag="v", name=f"cT{li}_{b}")
            nc.gpsimd.tensor_scalar(cT, negT, -1.0, 0.0, op0=MUL, op1=ADD)
            consts = (lambda: cR, lambda: cT, lambda: negT)
            return consts, cT

        # prologue: pos(0) interleaved with L0(0) in chunk-ready order
        st0 = new_batch_state(0)
        load_wpos_rest()
        load_wm()
        states = {0: st0}
        p0 = pos_tasks(0, st0[0], st0[1])
        consts0 = mk_consts_L0()
        r1_0 = netp.tile([128, T], BF16, tag="net", name="r1_0")
        acc0_0 = mk_acc("a0_0")
        l0_0 = layer_tasks(0, 0, st0[1], None, r1_0, acc0_0, consts0)
        for t in p0[0:4]:
            t()
        l0_0[0](); l0_0[2]()
        for t in p0[4:8]:
            t()
        l0_0[1](); l0_0[3]()
        for t in p0[8:12]:
            t()
        l0_0[4](); l0_0[6]()
        for t in p0[12:16]:
            t()
        l0_0[5](); l0_0[7]()

        for b in range(BPC):
            _, r0 = states[b]

            if b == 0:
                r1, acc0 = r1_0, acc0_0
            else:
                consts0 = mk_consts_L0()
                r1 = netp.tile([128, T], BF16, tag="net", name=f"r1_{b}")
                acc0 = mk_acc(f"a0_{b}")
                for t in layer_tasks(0, b, r0, None, r1, acc0, consts0):
                    t()

            filler = []
            if b + 1 < BPC:
                stn = new_batch_state(b + 1)
                states[b + 1] = stn
                filler = pos_tasks(b + 1, stn[0], stn[1])
            for t in filler[0:2]:
                t()
            consts1, cT1 = boundary(0, b, acc0, consts0)

            r2 = netp.tile([128, T], BF16, tag="net", name=f"r2_{b}")
            acc1 = mk_acc(f"a1_{b}")
            for t in interleave(layer_tasks(1, b, None, r1, r2, acc1, consts1),
                                filler[2:5]):
                t()
            consts2, cT2 = boundary(1, b, acc1, consts1)

            r3 = netp.tile([128, T], BF16, tag="net", name=f"r3_{b}")
            acc2 = mk_acc(f"a2_{b}")
            for t in interleave(layer_tasks(2, b, None, r2, r3, acc2, consts2),
                                filler[5:10]):
                t()
            constsD, cTD = boundary(2, b, acc2, consts2)
            cRD = constsD[0]()

            accD = mk_acc(f"aD_{b}")
            dts, (rcols, tcols) = d_tasks(b, r3, accD, last=(b + 1 == BPC))
            for t in interleave(dts, filler[10:16]):
                t()

            # s = relu(max(reduce(accD_T) + C_T^D, reduce(accD_R) + C_R^D))
            # (all DVE: engine-internal ordering, no cross-engine sem hops)
            sR0 = vecp.tile([128, 1], F32, tag="v", name=f"sR0_{b}")
            nc.vector.tensor_reduce(sR0, accD[:, rcols[0]:rcols[1]], AX, MAX)
            sRc = vecp.tile([128, 1], F32, tag="v", name=f"sRc_{b}")
            nc.vector.tensor_scalar(sRc, sR0, cRD, 0.0, op0=ADD, op1=ADD)
            sT0 = vecp.tile([128, 1], F32, tag="v", name=f"sT0_{b}")
            nc.vector.tensor_reduce(sT0, accD[:, tcols[0]:tcols[1]], AX, MAX)
            spre = vecp.tile([128, 1], F32, tag="v", name=f"sp_{b}")
            nc.vector.scalar_tensor_tensor(spre, sT0, cTD, sRc,
                                           op0=ADD, op1=MAX)
            s_b = vecp.tile([128, 1], F32, tag="v", name=f"s_{b}")
            nc.vector.tensor_scalar(s_b, spre, 0.0, 0.0, op0=MAX, op1=ADD)

            # incremental head for this batch (all tiny fp32 ops)
            hb = s_b
            for wi, bi in ((8, BC_BC), (9, BC_BM0), (10, BC_BM1), (11, BC_BM2)):
                ps = psvp.tile([128, 1], F32, tag="psv", name=f"ph{wi}_{b}")
                nc.tensor.matmul(ps[:], Wf(wi), hb[:], start=True, stop=True)
                h2 = vecp.tile([128, 1], F32, tag="v", name=f"h{wi}_{b}")
                nc.scalar.activation(h2, ps[:], RELU, bias=bcol(bi))
                hb = h2
            ps9 = psvp.tile([9, 1], F32, tag="psv", name=f"po_{b}")
            nc.tensor.matmul(ps9[:], wmf_sb[:, 1536:1536 + 9], hb[:],
                             start=True, stop=True)
            ob = headp.tile([9, 1], F32, tag="o", name=f"ob_{b}")
            nc.scalar.activation(ob, ps9[:], IDENT, bias=bias_sb[0:9, BC_BP:BC_BP + 1])
            nc.sync.dma_start(out_d[b:b + 1].rearrange("b f -> f b"), ob[:])

    for p in reversed(ctx_pools):
        p.release()


def build_program(reps=1):
    nc = bacc.Bacc("TRN2", target_bir_lowering=False, debug=False,
                   num_devices=NCORES)
    pt_d = nc.dram_tensor("pt", [BPC, NQ, 3, QT], BF16, kind="ExternalInput").ap()
    wpos_d = nc.dram_tensor("wpos", [NQ, 3, 256], BF16, kind="ExternalInput").ap()
    wm16_d = nc.dram_tensor("wm16", [128, WM_COLS], BF16, kind="ExternalInput").ap()
    wmf_d = nc.dram_tensor("wmf", [128, WM_COLS], F32, kind="ExternalInput").ap()
    wmn_d = nc.dram_tensor("wmn", [128, 384], F32, kind="ExternalInput").ap()
    bias_d = nc.dram_tensor("bias", [128, 16], F32, kind="ExternalInput").ap()
    out_d = nc.dram_tensor("out", [BPC, 9], F32, kind="ExternalOutput").ap()
    with tile.TileContext(nc) as tc:
        _emit_core_program(tc, nc, pt_d, wpos_d, wm16_d, wmf_d, wmn_d,
                           bias_d, out_d, reps=reps)
    nc.compile()
    return nc


def prepare_host_inputs(inputs):
    """Shared (weights) and per-core (points) host-side packing."""
    import ml_dtypes
    BF = ml_dtypes.bfloat16
    f = lambda k: np.ascontiguousarray(np.asarray(inputs[k], np.float32))
    p = f("p")
    W_pos, b_pos = f("W_pos"), f("b_pos")
    W0, b0 = f("W0"), f("b0")
    W1, b1 = f("W1"), f("b1")
    W2, b2 = f("W2"), f("b2")
    W3, b3 = f("W3"), f("b3")
    Wc, bc = f("Wc"), f("bc")
    Wm0, bm0 = f("Wm0"), f("bm0")
    Wm1, bm1 = f("Wm1"), f("bm1")
    Wm2, bm2 = f("Wm2"), f("bm2")
    Wp, bp = f("Wp"), f("bp")

    wpos = np.broadcast_to(W_pos, (NQ, 3, 256)).copy()  # replicated per quad

    wm = np.zeros((128, WM_COLS), np.float32)
    blocks = [W0[:128], W0[128:], W1[:128], W1[128:], W2[:128], W2[128:],
              W3[:128], W3[128:], Wc, Wm0, Wm1, Wm2]
    for i, blk in enumerate(blocks):
        wm[:, 128 * i:128 * (i + 1)] = blk
    wm[:, 1536:1536 + 9] = Wp

    bias = np.zeros((128, 16), np.float32)
    bias[:, BC_BPOS_A] = b_pos[:128]
    bias[:, BC_BPOS_B] = b_pos[128:]
    bias[:, BC_B0] = b0
    bias[:, BC_B1] = b1
    bias[:, BC_B2] = b2
    bias[:, BC_B3] = b3
    bias[:, BC_BC] = bc
    bias[:, BC_BM0] = bm0
    bias[:, BC_BM1] = bm1
    bias[:, BC_BM2] = bm2
    bias[:9, BC_BP] = bp
    bias[:, BC_NEG_B0] = -b0
    bias[:, BC_NEG_B1] = -b1
    bias[:, BC_NEG_B2] = -b2
    bias[:, BC_NEG_B3] = -b3

    wmn = -np.concatenate([W1[128:], W2[128:], W3[128:]], axis=1)
    shared = {"wpos": wpos.astype(BF), "wm16": wm.astype(BF),
              "wmf": wm, "wmn": np.ascontiguousarray(wmn), "bias": bias}

    in_maps = []
    for core in range(NCORES):
        pc = p[core * BPC:(core + 1) * BPC]          # [BPC, T, 3]
        pt = np.empty((BPC, NQ, 3, QT), np.float32)
        for b in range(BPC):
            for q in range(NQ):
                pt[b, q] = pc[b, q * QT:(q + 1) * QT, :].T
        in_maps.append({"pt": pt.astype(BF), **shared})
    return in_maps


_PROGRAM_CACHE = {}


def kernel(**inputs):
    reps = 1
    if reps not in _PROGRAM_CACHE:
        _PROGRAM_CACHE[reps] = build_program(reps)
    nc = _PROGRAM_CACHE[reps]
    in_maps = prepare_host_inputs(inputs)
    res = run_bass_kernel_spmd(nc, in_maps, core_ids=list(range(NCORES)))
    out = np.concatenate([res.results[i]["out"] for i in range(NCORES)], axis=0)
    return out.astype(np.float32)
